# revision 1
# baseline (speedup 1.0000x reference)
"""GCN cascade layer (3 parallel GCNConv + 1 linear head) on 8 Trainium2 cores.

Math (per edge set i):
    deg[c]   = sum_{e: col=c} w[e]
    dinv     = deg>0 ? 1/sqrt(deg) : 0
    h        = x @ W_i.T
    out[c]   = relu( sum_e dinv[row]*w*dinv[c] * h[row] + b )
Reformulated so per-edge work is gather + weighted scatter only:
    h'[r]    = h[r] * dinv_m[r]            (dinv_m masked: 0 where deg==0)
    t[c]     = sum_{e->c} w[e] * h'[row[e]]
    out[c]   = relu( dinv_s[c] * (t[c] + s2[c]*b) )
  where dinv_s = 1/sqrt(max(deg,1-ish)) (==1 at deg==0), s2 = 1/dinv_s,
  so dinv_s*s2 == 1 and deg==0 columns give relu(b) exactly like the ref.

Distribution: output columns (nodes) sharded over 8 cores in contiguous
128-col tiles; h' computed replicated on every core (one pass over x for all
3 sets); per-core edges bucketed by 128-col tile on the host; gather of
h'[row] via the custom SWDGE dma_gather; scatter = one-hot-weighted matmul
accumulating into PSUM (exact segment sum on the TensorEngine).
"""

import sys

sys.path.insert(0, "/opt/trn_rl_repo")

import math
from dataclasses import dataclass, field

import numpy as np

import concourse.bass as bass
import concourse.bacc as bacc
import concourse.mybir as mybir
from concourse import tile, library_config

P = 128          # partitions / feature dim
CORES = 8
GATHER_GROUP = 7  # node tiles per dma_gather call group

f16 = mybir.dt.float16
f32 = mybir.dt.float32
i16 = mybir.dt.int16

# set by run(): BassKernelResults of the last hardware run (for profiling)
LAST_RESULTS = None
TRACE = False


@dataclass
class Cfg:
    N: int
    E: int
    A: int
    TPC: int          # node tiles per core
    NT: int           # total node tiles (CORES*TPC)
    N2: int           # padded node count (NT*P)
    OWN: int          # cols/rows owned per core (TPC*P)
    LO_T: int         # tiles in the "lo" half of h'
    LO_ROWS: int
    HI_T: int
    HI_ROWS: int
    S: int            # padded max col-degree slots
    K_lo: int         # lo chunks per node tile
    K_hi: int
    K_u: int          # K_lo + K_hi
    n_groups: int = 0
    group_tiles: list = field(default_factory=list)


def _make_cfg(N, E, A):
    TPC = math.ceil(N / (CORES * P))
    NT = CORES * TPC
    N2 = NT * P
    LO_T = (NT + 1) // 2
    LO_ROWS = LO_T * P
    HI_T = NT - LO_T
    HI_ROWS = HI_T * P
    assert LO_ROWS < 32768 and HI_ROWS < 32768, "int16 gather index overflow"
    cfg = Cfg(N=N, E=E, A=A, TPC=TPC, NT=NT, N2=N2, OWN=TPC * P,
              LO_T=LO_T, LO_ROWS=LO_ROWS, HI_T=HI_T, HI_ROWS=HI_ROWS,
              S=0, K_lo=0, K_hi=0, K_u=0)
    g = min(GATHER_GROUP, TPC)
    cfg.n_groups = math.ceil(TPC / g)
    cfg.group_tiles = [min(g, TPC - i * g) for i in range(cfg.n_groups)]
    return cfg


def _prep(cfg, x, edge_index, edge_attr, lin_w, lin_b, conv_w, conv_b):
    """Host-side sharding/layout prep. Returns (in_maps, aux) where in_maps is
    the per-core input dict list."""
    A, N, E = cfg.A, cfg.N, cfg.E
    TPC, NT, S0 = cfg.TPC, cfg.NT, None

    r_all = edge_index[:, 0, :].astype(np.int64)   # [A,E]
    c_all = edge_index[:, 1, :].astype(np.int64)
    w_all = edge_attr.astype(np.float32)

    # --- degree (host only for LAYOUT: slot ranks + max degree) ---
    # padded per-col weight layout for the on-device degree reduce
    deg_counts = np.zeros((A, cfg.N2), np.int64)
    for i in range(A):
        deg_counts[i] = np.bincount(c_all[i], minlength=cfg.N2)
    S = int(deg_counts.max())
    S = max(S, 1)
    cfg.S = S

    # wdeg_all[i, p, tglob*S + s] = s-th weight of col (tglob*P + p)
    wdeg_all = np.zeros((A, P, NT * S), np.float16)
    # wdeg_own[k][i, p, tloc*S + s] : own cols only
    wdeg_own = np.zeros((CORES, A, P, TPC * S), np.float16)

    # --- edge bucketing ---
    # per (set, tile, half): edge lists; uniform chunk counts across cores.
    K_lo = K_hi = 0
    per_set = []
    for i in range(A):
        c = c_all[i]
        r = r_all[i]
        w = w_all[i]
        tile_of = c // P
        is_hi = (r >= cfg.LO_ROWS).astype(np.int64)
        # sort edges by (tile, half); stable order within
        order = np.lexsort((is_hi, tile_of))
        c_s, r_s, w_s, t_s, hi_s = c[order], r[order], w[order], tile_of[order], is_hi[order]
        # rank within each (tile, half) segment
        seg_key = t_s * 2 + hi_s
        seg_change = np.empty(E, np.bool_)
        seg_change[0] = True
        seg_change[1:] = seg_key[1:] != seg_key[:-1]
        seg_start_idx = np.flatnonzero(seg_change)
        starts = np.zeros(E, np.int64)
        starts[seg_start_idx] = seg_start_idx
        starts = np.maximum.accumulate(starts)
        rank = np.arange(E) - starts
        # counts per (tile, half)
        n_lo = np.bincount(t_s[hi_s == 0], minlength=NT)
        n_hi = np.bincount(t_s[hi_s == 1], minlength=NT)
        K_lo = max(K_lo, int(math.ceil(n_lo.max() / P)))
        K_hi = max(K_hi, int(math.ceil(n_hi.max() / P)))
        per_set.append((c_s, r_s, w_s, t_s, hi_s, rank))

        # degree slot layout
        csort = np.sort(c)
        crank = np.arange(E) - np.maximum.accumulate(
            np.where(np.r_[True, csort[1:] != csort[:-1]], np.arange(E), 0))
        p_of = csort % P
        t_of = csort // P
        # values: need w ordered the same as csort
        worder = np.argsort(c, kind="stable")
        wdeg_all[i, p_of, t_of * S + crank] = w[worder].astype(np.float16)
        for k in range(CORES):
            sel = (t_of >= k * TPC) & (t_of < (k + 1) * TPC)
            wdeg_own[k, i, p_of[sel], (t_of[sel] - k * TPC) * S + crank[sel]] = \
                w[worder][sel].astype(np.float16)

    K_lo = max(K_lo, 1)
    K_hi = max(K_hi, 1) if cfg.HI_T > 0 else 0
    cfg.K_lo, cfg.K_hi, cfg.K_u = K_lo, K_hi, K_lo + K_hi

    # --- per-core metadata arrays ---
    CH = TPC * cfg.K_u                     # chunks per core per set
    colloc = np.zeros((CORES, A, P, CH), np.float32)
    wchunk = np.zeros((CORES, A, P, CH), np.float32)
    gidx_lo = np.zeros((CORES, A, 16, TPC * K_lo * 8), np.int16)
    gidx_hi = np.zeros((CORES, A, 16, max(TPC * K_hi * 8, 1)), np.int16)

    for i in range(A):
        c_s, r_s, w_s, t_s, hi_s, rank = per_set[i]
        core = t_s // TPC
        tloc = t_s % TPC
        kk = rank // P          # chunk index within (tile, half)
        jj = rank % P           # partition
        lo_m = hi_s == 0
        # chunk column in colloc/wchunk: tloc*K_u + kk (lo) / + K_lo + kk (hi)
        col_idx = np.where(lo_m, tloc * cfg.K_u + kk, tloc * cfg.K_u + K_lo + kk)
        colloc[core, i, jj, col_idx] = (c_s % P).astype(np.float32)
        wchunk[core, i, jj, col_idx] = w_s.astype(np.float32)
        # gather sequence position: tile-major within lo/hi streams
        gi = np.where(lo_m, r_s, r_s - cfg.LO_ROWS).astype(np.int16)
        pos = tloc * (np.where(lo_m, K_lo, K_hi) * P) + rank
        # scatter into [16, L/16] interleaved layout: seq idx q -> [q%16, q//16]
        lo_sel = lo_m
        gidx_lo[core[lo_sel], i, pos[lo_sel] % 16, pos[lo_sel] // 16] = gi[lo_sel]
        if cfg.HI_T > 0:
            hi_sel = ~lo_m
            gidx_hi[core[hi_sel], i, pos[hi_sel] % 16, pos[hi_sel] // 16] = gi[hi_sel]

    # --- dense-phase inputs ---
    xpad = np.zeros((cfg.N2, P), np.float32)
    xpad[:N] = x
    xT_all = np.ascontiguousarray(xpad.T).astype(np.float16)        # [P, N2]
    WT = np.ascontiguousarray(conv_w.transpose(0, 2, 1)).astype(np.float16)  # [A,P,P]
    linWT = np.ascontiguousarray(lin_w.T).astype(np.float16)        # [P,P]
    lin_b_row = lin_b.reshape(1, P).astype(np.float16)
    b_rows = conv_b.reshape(A, 1, P).astype(np.float16)
    iota_row = np.tile(np.arange(P, dtype=np.float16), (P, 1))       # [P,P]
    ones_row = np.ones((1, P), np.float16)
    pidx_col = np.arange(P, dtype=np.float32).reshape(P, 1)           # [P,1]
    b_bcast = np.tile(conv_b.reshape(A, 1, P), (1, P, 1)).astype(np.float16)  # [A,P,P]

    in_maps = []
    for k in range(CORES):
        m = dict(
            xT_all=xT_all,
            xT_own=np.ascontiguousarray(xT_all[:, k * cfg.OWN:(k + 1) * cfg.OWN]),
            wdeg_all=wdeg_all,
            wdeg_own=wdeg_own[k],
            WT=WT, linWT=linWT, lin_b_row=lin_b_row, b_rows=b_rows,
            iota_row=iota_row, ones_row=ones_row,
            pidx_col=pidx_col, b_bcast=b_bcast,
            colloc=colloc[k], wchunk=wchunk[k],
            gidx_lo=np.tile(gidx_lo[k], (1, 8, 1)),
        )
        if cfg.HI_T > 0:
            m["gidx_hi"] = np.tile(gidx_hi[k], (1, 8, 1))
        in_maps.append(m)
    return in_maps


def _build(cfg):
    """Build the single SPMD Bass program."""
    nc = bacc.Bacc()
    A, TPC, NT, S = cfg.A, cfg.TPC, cfg.NT, cfg.S
    K_lo, K_hi, K_u = cfg.K_lo, cfg.K_hi, cfg.K_u
    CH = TPC * K_u
    Alu = mybir.AluOpType
    Act = mybir.ActivationFunctionType

    # ---- I/O ----
    xT_all = nc.dram_tensor("xT_all", [P, cfg.N2], f16, kind="ExternalInput")
    xT_own = nc.dram_tensor("xT_own", [P, cfg.OWN], f16, kind="ExternalInput")
    wdeg_all = nc.dram_tensor("wdeg_all", [A, P, NT * S], f16, kind="ExternalInput")
    wdeg_own = nc.dram_tensor("wdeg_own", [A, P, TPC * S], f16, kind="ExternalInput")
    WT = nc.dram_tensor("WT", [A, P, P], f16, kind="ExternalInput")
    linWT = nc.dram_tensor("linWT", [P, P], f16, kind="ExternalInput")
    lin_b_row = nc.dram_tensor("lin_b_row", [1, P], f16, kind="ExternalInput")
    b_rows = nc.dram_tensor("b_rows", [A, 1, P], f16, kind="ExternalInput")
    iota_row = nc.dram_tensor("iota_row", [P, P], f16, kind="ExternalInput")
    ones_row = nc.dram_tensor("ones_row", [1, P], f16, kind="ExternalInput")
    pidx_col = nc.dram_tensor("pidx_col", [P, 1], f32, kind="ExternalInput")
    b_bcast = nc.dram_tensor("b_bcast", [A, P, P], f16, kind="ExternalInput")
    colloc_d = nc.dram_tensor("colloc", [A, P, CH], f32, kind="ExternalInput")
    wchunk_d = nc.dram_tensor("wchunk", [A, P, CH], f32, kind="ExternalInput")
    gidx_lo_d = nc.dram_tensor("gidx_lo", [A, 128, TPC * K_lo * 8], i16,
                               kind="ExternalInput")
    gidx_hi_d = (nc.dram_tensor("gidx_hi", [A, 128, TPC * K_hi * 8], i16,
                                kind="ExternalInput") if cfg.HI_T > 0 else None)

    hs0 = nc.dram_tensor("hs0", [cfg.OWN, P], f32, kind="ExternalOutput")
    outs = [nc.dram_tensor(f"out{i}", [cfg.OWN, P], f32, kind="ExternalOutput")
            for i in range(A)]

    h_lo = [nc.dram_tensor(f"h{i}_lo", [cfg.LO_ROWS, P], f16) for i in range(A)]
    h_hi = [nc.dram_tensor(f"h{i}_hi", [max(cfg.HI_ROWS, P), P], f16)
            for i in range(A)]

    with tile.TileContext(nc) as tc:
        with (
            tc.tile_pool(name="const", bufs=1) as cpool,
            tc.tile_pool(name="meta", bufs=1) as mpool,
            tc.tile_pool(name="degio", bufs=2) as dpool,
            tc.tile_pool(name="degres", bufs=1) as rpool,
            tc.tile_pool(name="xw", bufs=3) as xpool,
            tc.tile_pool(name="hstage", bufs=4) as hpool,
            tc.tile_pool(name="glo", bufs=2) as glo_pool,
            tc.tile_pool(name="ghi", bufs=2) as ghi_pool,
            tc.tile_pool(name="gix", bufs=2) as gix_pool,
            tc.tile_pool(name="bw", bufs=4) as bwpool,
            tc.tile_pool(name="outst", bufs=3) as opool,
            tc.tile_pool(name="psd", bufs=3, space="PSUM") as psd,
            tc.tile_pool(name="pss", bufs=3, space="PSUM") as pss,
        ):
            # ---- constants to SBUF ----
            iota_t = cpool.tile([P, P], f16)
            nc.sync.dma_start(out=iota_t[:], in_=iota_row[:])
            ones_t = cpool.tile([1, P], f16)
            nc.sync.dma_start(out=ones_t[:], in_=ones_row[:])
            linb_t = cpool.tile([1, P], f16)
            nc.sync.dma_start(out=linb_t[:], in_=lin_b_row[:])
            linWT_t = cpool.tile([P, P], f16)
            nc.sync.dma_start(out=linWT_t[:], in_=linWT[:])
            pidx_t = cpool.tile([P, 1], f32)
            nc.sync.dma_start(out=pidx_t[:], in_=pidx_col[:])
            WT_t = []
            b_t = []
            for i in range(A):
                wt = cpool.tile([P, P], f16, tag=f"WT{i}")
                nc.sync.dma_start(out=wt[:], in_=WT[i, :, :])
                WT_t.append(wt)
                bt = cpool.tile([P, P], f16, tag=f"bt{i}")
                nc.sync.dma_start(out=bt[:], in_=b_bcast[i, :, :])
                b_t.append(bt)
            colloc_t = []
            wchunk_t = []
            for i in range(A):
                ct = mpool.tile([P, CH], f32, tag=f"colloc{i}")
                nc.sync.dma_start(out=ct[:], in_=colloc_d[i, :, :])
                colloc_t.append(ct)
                wt = mpool.tile([P, CH], f32, tag=f"wchunk{i}")
                nc.sync.dma_start(out=wt[:], in_=wchunk_d[i, :, :])
                wchunk_t.append(wt)

            # ---- degree phase ----
            def deg_pipeline(src, n_tiles, tag):
                """reduce padded weights [P, n_tiles*S] -> deg [P,n_tiles] f32,
                returns (m, dinv_masked, dinv_safe) resident tiles."""
                deg = rpool.tile([P, n_tiles], f32, tag=f"deg_{tag}")
                GRP = max(1, min(n_tiles, 4096 // S))
                for g0 in range(0, n_tiles, GRP):
                    g1 = min(n_tiles, g0 + GRP)
                    wt = dpool.tile([P, GRP * S], f16, tag="degload")
                    nc.sync.dma_start(out=wt[:, :(g1 - g0) * S],
                                      in_=src[:, g0 * S:g1 * S])
                    nc.vector.tensor_reduce(
                        out=deg[:, g0:g1],
                        in_=wt[:, :(g1 - g0) * S].rearrange("p (g s) -> p g s", s=S),
                        axis=mybir.AxisListType.X, op=Alu.add)
                m = rpool.tile([P, n_tiles], f32, tag=f"m_{tag}")
                nc.vector.tensor_scalar(out=m[:], in0=deg[:], scalar1=0.0,
                                        scalar2=None, op0=Alu.is_gt)
                degsafe = rpool.tile([P, n_tiles], f32, tag=f"ds_{tag}")
                nc.vector.tensor_scalar(out=degsafe[:], in0=deg[:], scalar1=1.0,
                                        scalar2=None, op0=Alu.add)
                nc.vector.tensor_tensor(out=degsafe[:], in0=degsafe[:], in1=m[:],
                                        op=Alu.subtract)
                s = rpool.tile([P, n_tiles], f32, tag=f"s_{tag}")
                nc.scalar.activation(out=s[:], in_=degsafe[:], func=Act.Sqrt)
                dinv_safe = rpool.tile([P, n_tiles], f32, tag=f"dvs_{tag}")
                nc.vector.reciprocal(out=dinv_safe[:], in_=s[:])
                dinv_m = rpool.tile([P, n_tiles], f32, tag=f"dvm_{tag}")
                nc.vector.tensor_tensor(out=dinv_m[:], in0=dinv_safe[:], in1=m[:],
                                        op=Alu.mult)
                return m, dinv_m, dinv_safe, s

            dinv_row = []   # [P, NT] f32 masked, per set
            dinv_col = []   # [P, TPC] f32 safe, per set
            s_col = []      # [P, TPC] f32 safe sqrt(deg), per set
            for i in range(A):
                _, dm, _, _ = deg_pipeline(wdeg_all[i, :, :], NT, f"all{i}")
                dinv_row.append(dm)
                _, _, dvs, s_own = deg_pipeline(wdeg_own[i, :, :], TPC, f"own{i}")
                dinv_col.append(dvs)
                s_col.append(s_own)

            # ---- dense phase: h'_i (replicated over all NT tiles) ----
            for t in range(NT):
                xt = xpool.tile([P, P], f16, tag="xt")
                nc.sync.dma_start(out=xt[:], in_=xT_all[:, t * P:(t + 1) * P])
                for i in range(A):
                    ps = psd.tile([P, P], f32, tag="psd")
                    nc.tensor.matmul(out=ps[:], lhsT=xt[:], rhs=WT_t[i][:],
                                     start=True, stop=True)
                    hh = hpool.tile([P, P], f16, tag="hh")
                    nc.scalar.activation(out=hh[:], in_=ps[:], func=Act.Copy,
                                         scale=dinv_row[i][:, t:t + 1])
                    if t < cfg.LO_T:
                        dst = h_lo[i][t * P:(t + 1) * P, :]
                    else:
                        tt = t - cfg.LO_T
                        dst = h_hi[i][tt * P:(tt + 1) * P, :]
                    nc.sync.dma_start(out=dst, in_=hh[:])

            # ---- hs0 (own rows only) ----
            for t in range(TPC):
                xo = xpool.tile([P, P], f16, tag="xo")
                nc.sync.dma_start(out=xo[:], in_=xT_own[:, t * P:(t + 1) * P])
                ps = psd.tile([P, P], f32, tag="psd")
                nc.tensor.matmul(out=ps[:], lhsT=xo[:], rhs=linWT_t[:],
                                 start=True, stop=False)
                nc.tensor.matmul(out=ps[:], lhsT=ones_t[:], rhs=linb_t[:],
                                 start=False, stop=True)
                ob = opool.tile([P, P], f32, tag="ob")
                nc.scalar.activation(out=ob[:], in_=ps[:], func=Act.Relu)
                nc.sync.dma_start(out=hs0[t * P:(t + 1) * P, :], in_=ob[:])

            # ---- scatter phase per set ----
            for i in range(A):
                hlo_view = h_lo[i][:, :]
                hhi_view = h_hi[i][:, :]
                for g in range(cfg.n_groups):
                    tg = cfg.group_tiles[g]
                    t0 = g * min(GATHER_GROUP, TPC)
                    GT = min(GATHER_GROUP, TPC)
                    # lo gather
                    glo = glo_pool.tile([P, GT * K_lo, P], f16, tag="glo")
                    L = tg * K_lo * P
                    gixt = gix_pool.tile([128, GT * K_lo * 8], i16, tag="gixlo")
                    off = t0 * K_lo * 8
                    nc.sync.dma_start(out=gixt[:, :L // 16],
                                      in_=gidx_lo_d[i, :, off:off + L // 16])
                    nc.gpsimd.dma_gather(
                        out_ap=glo[:, :tg * K_lo, :], in_ap=hlo_view,
                        idxs_ap=gixt[:, :L // 16],
                        num_idxs=L, num_idxs_reg=L, elem_size=P,
                        single_packet=False)
                    # hi gather
                    if cfg.HI_T > 0:
                        ghi = ghi_pool.tile([P, GT * K_hi, P], f16, tag="ghi")
                        Lh = tg * K_hi * P
                        gixh = gix_pool.tile([128, GT * K_hi * 8], i16, tag="gixhi")
                        offh = t0 * K_hi * 8
                        nc.sync.dma_start(out=gixh[:, :Lh // 16],
                                          in_=gidx_hi_d[i, :, offh:offh + Lh // 16])
                        nc.gpsimd.dma_gather(
                            out_ap=ghi[:, :tg * K_hi, :], in_ap=hhi_view,
                            idxs_ap=gixh[:, :Lh // 16],
                            num_idxs=Lh, num_idxs_reg=Lh, elem_size=P,
                            single_packet=False)
                    for tl in range(tg):
                        tt = t0 + tl
                        ps = pss.tile([P, P], f32, tag="pss")
                        for k in range(K_u):
                            gc = tt * K_u + k
                            bw = bwpool.tile([P, P], f16, tag="bw")
                            nc.vector.tensor_scalar(
                                out=bw[:], in0=iota_t[:],
                                scalar1=colloc_t[i][:, gc:gc + 1],
                                scalar2=wchunk_t[i][:, gc:gc + 1],
                                op0=Alu.is_equal, op1=Alu.mult)
                            if k < K_lo:
                                src = glo[:, tl * K_lo + k, :]
                            else:
                                src = ghi[:, tl * K_hi + (k - K_lo), :]
                            nc.tensor.matmul(out=ps[:], lhsT=bw[:], rhs=src,
                                             start=(k == 0), stop=False)
                        # bias: += diag(s2) @ b_bcast  ==> (t + s2*b)
                        dg = bwpool.tile([P, P], f16, tag="dg")
                        nc.vector.tensor_scalar(
                            out=dg[:], in0=iota_t[:],
                            scalar1=pidx_t[:, 0:1],
                            scalar2=s_col[i][:, tt:tt + 1],
                            op0=Alu.is_equal, op1=Alu.mult)
                        nc.tensor.matmul(out=ps[:], lhsT=dg[:], rhs=b_t[i][:],
                                         start=False, stop=True)
                        ot = opool.tile([P, P], f32, tag="ot")
                        nc.vector.tensor_scalar(
                            out=ot[:], in0=ps[:],
                            scalar1=dinv_col[i][:, tt:tt + 1],
                            scalar2=0.0, op0=Alu.mult, op1=Alu.max)
                        nc.sync.dma_start(out=outs[i][tt * P:(tt + 1) * P, :],
                                          in_=ot[:])
    nc.finalize()
    return nc


def _assemble(cfg, results):
    """results: list of per-core output dicts -> full outputs tuple."""
    N, A = cfg.N, cfg.A
    hs = []
    h0 = np.concatenate([results[k]["hs0"] for k in range(CORES)], axis=0)[:N]
    hs.append(h0.astype(np.float32))
    for i in range(A):
        o = np.concatenate([results[k][f"out{i}"] for k in range(CORES)], axis=0)[:N]
        hs.append(o.astype(np.float32))
    return tuple(hs)


def kernel(x, edge_index, edge_attr, lin_w, lin_b, conv_w, conv_b):
    global LAST_RESULTS
    x = np.asarray(x, np.float32)
    edge_index = np.asarray(edge_index)
    edge_attr = np.asarray(edge_attr, np.float32)
    lin_w = np.asarray(lin_w, np.float32)
    lin_b = np.asarray(lin_b, np.float32)
    conv_w = np.asarray(conv_w, np.float32)
    conv_b = np.asarray(conv_b, np.float32)

    N, D = x.shape
    A, _, E = edge_index.shape
    assert D == P
    cfg = _make_cfg(N, E, A)
    in_maps = _prep(cfg, x, edge_index, edge_attr, lin_w, lin_b, conv_w, conv_b)
    nc = _build(cfg)

    from concourse.bass_utils import run_bass_kernel_spmd
    res = run_bass_kernel_spmd(nc, in_maps, list(range(CORES)), trace=TRACE)
    LAST_RESULTS = res
    return _assemble(cfg, res.results)


# ---------- simulation path (for testing on small configs) ----------

def run_sim(x, edge_index, edge_attr, lin_w, lin_b, conv_w, conv_b,
            cores=None):
    """Run each core through CoreSim; returns assembled outputs."""
    from concourse import bass_interp
    x = np.asarray(x, np.float32)
    edge_index = np.asarray(edge_index)
    edge_attr = np.asarray(edge_attr, np.float32)
    N, D = x.shape
    A, _, E = edge_index.shape
    cfg = _make_cfg(N, E, A)
    in_maps = _prep(cfg, x, edge_index, edge_attr,
                    np.asarray(lin_w, np.float32), np.asarray(lin_b, np.float32),
                    np.asarray(conv_w, np.float32), np.asarray(conv_b, np.float32))
    results = []
    for k in (range(CORES) if cores is None else cores):
        nc = _build(cfg)
        sim = bass_interp.CoreSim(nc, core_id=0)
        sim.assign_tensors(in_maps[k])
        sim.simulate()
        results.append({name: sim.tensor(name).copy()
                        for name in ["hs0"] + [f"out{i}" for i in range(A)]})
    if cores is not None:
        return cfg, results
    return _assemble(cfg, results)



# revision 2
# speedup vs baseline: 1.3708x; 1.3708x over previous
"""GCN cascade layer (3 parallel GCNConv + 1 linear head) on 8 Trainium2 cores.

Math (per edge set i):
    deg[c]   = sum_{e: col=c} w[e]
    dinv     = deg>0 ? 1/sqrt(deg) : 0
    h        = x @ W_i.T
    out[c]   = relu( sum_e dinv[row]*w*dinv[c] * h[row] + b )
Reformulated so per-edge work is gather + weighted scatter only:
    h'[r]    = h[r] * dinv_m[r]            (dinv_m masked: 0 where deg==0)
    t[c]     = sum_{e->c} w[e] * h'[row[e]]
    out[c]   = relu( dinv_s[c] * (t[c] + s2[c]*b) )
  where dinv_s = 1/sqrt(max(deg,1-ish)) (==1 at deg==0), s2 = 1/dinv_s,
  so dinv_s*s2 == 1 and deg==0 columns give relu(b) exactly like the ref.

Distribution: output columns (nodes) sharded over 8 cores in contiguous
128-col tiles; h' computed replicated on every core (one pass over x per
set); per-core edges bucketed by 128-col tile on the host; gather of
h'[row] via the custom SWDGE dma_gather; scatter = one-hot-weighted matmul
accumulating into PSUM (exact segment sum on the TensorEngine).

Perf structure (v2): dma_gather descriptor generation runs on Q7 core pair
[2q, 2q+1] selected by queue_num — with num_swdge_queues=4 and round-robin
queue assignment, four gathers generate descriptors concurrently. Emission
is pipelined (dense_0, dense_1, edge_0, dense_2, edge_1, edge_2) so the PE
has dense work queued while the first gathers run, and DMAs are batched
4 tiles per transfer to cut sync-engine issue overhead.
"""

import sys

sys.path.insert(0, "/opt/trn_rl_repo")

import math
from dataclasses import dataclass, field

import numpy as np

import concourse.bass as bass
import concourse.bacc as bacc
import concourse.mybir as mybir
from concourse import tile, library_config

P = 128          # partitions / feature dim
CORES = 8
GATHER_GROUP = 7  # node tiles per dma_gather call group
NQ = 4            # SWDGE queues (gather desc-gen concurrency)
TB = 4            # tiles per batched DMA

f16 = mybir.dt.float16
f32 = mybir.dt.float32
i16 = mybir.dt.int16

# set by run(): BassKernelResults of the last hardware run (for profiling)
LAST_RESULTS = None
TRACE = False


@dataclass
class Cfg:
    N: int
    E: int
    A: int
    TPC: int          # node tiles per core
    NT: int           # total node tiles (CORES*TPC)
    N2: int           # padded node count (NT*P)
    OWN: int          # cols/rows owned per core (TPC*P)
    LO_T: int         # tiles in the "lo" half of h'
    LO_ROWS: int
    HI_T: int
    HI_ROWS: int
    S: int            # padded max col-degree slots
    K_lo: int         # lo chunks per node tile
    K_hi: int
    K_u: int          # K_lo + K_hi
    n_groups: int = 0
    group_tiles: list = field(default_factory=list)


def _make_cfg(N, E, A):
    TPC = math.ceil(N / (CORES * P))
    NT = CORES * TPC
    N2 = NT * P
    LO_T = (NT + 1) // 2
    LO_ROWS = LO_T * P
    HI_T = NT - LO_T
    HI_ROWS = HI_T * P
    assert LO_ROWS < 32768 and HI_ROWS < 32768, "int16 gather index overflow"
    cfg = Cfg(N=N, E=E, A=A, TPC=TPC, NT=NT, N2=N2, OWN=TPC * P,
              LO_T=LO_T, LO_ROWS=LO_ROWS, HI_T=HI_T, HI_ROWS=HI_ROWS,
              S=0, K_lo=0, K_hi=0, K_u=0)
    g = min(GATHER_GROUP, TPC)
    cfg.n_groups = math.ceil(TPC / g)
    cfg.group_tiles = [min(g, TPC - i * g) for i in range(cfg.n_groups)]
    return cfg


def _prep(cfg, x, edge_index, edge_attr, lin_w, lin_b, conv_w, conv_b):
    """Host-side sharding/layout prep. Returns the per-core input dict list."""
    A, N, E = cfg.A, cfg.N, cfg.E
    TPC, NT = cfg.TPC, cfg.NT

    r_all = edge_index[:, 0, :].astype(np.int64)   # [A,E]
    c_all = edge_index[:, 1, :].astype(np.int64)
    w_all = edge_attr.astype(np.float32)

    # --- degree (host only for LAYOUT: slot ranks + max degree) ---
    deg_counts = np.zeros((A, cfg.N2), np.int64)
    for i in range(A):
        deg_counts[i] = np.bincount(c_all[i], minlength=cfg.N2)
    S = int(deg_counts.max())
    S = max(S, 1)
    cfg.S = S

    # wdeg_all[i, p, tglob*S + s] = s-th weight of col (tglob*P + p)
    wdeg_all = np.zeros((A, P, NT * S), np.float16)
    wdeg_own = np.zeros((CORES, A, P, TPC * S), np.float16)

    # --- edge bucketing ---
    K_lo = K_hi = 0
    per_set = []
    for i in range(A):
        c = c_all[i]
        r = r_all[i]
        w = w_all[i]
        tile_of = c // P
        is_hi = (r >= cfg.LO_ROWS).astype(np.int64)
        order = np.lexsort((is_hi, tile_of))
        c_s, r_s, w_s, t_s, hi_s = c[order], r[order], w[order], tile_of[order], is_hi[order]
        seg_key = t_s * 2 + hi_s
        seg_change = np.empty(E, np.bool_)
        seg_change[0] = True
        seg_change[1:] = seg_key[1:] != seg_key[:-1]
        seg_start_idx = np.flatnonzero(seg_change)
        starts = np.zeros(E, np.int64)
        starts[seg_start_idx] = seg_start_idx
        starts = np.maximum.accumulate(starts)
        rank = np.arange(E) - starts
        n_lo = np.bincount(t_s[hi_s == 0], minlength=NT)
        n_hi = np.bincount(t_s[hi_s == 1], minlength=NT)
        K_lo = max(K_lo, int(math.ceil(n_lo.max() / P)))
        K_hi = max(K_hi, int(math.ceil(n_hi.max() / P)))
        per_set.append((c_s, r_s, w_s, t_s, hi_s, rank))

        # degree slot layout
        csort = np.sort(c)
        crank = np.arange(E) - np.maximum.accumulate(
            np.where(np.r_[True, csort[1:] != csort[:-1]], np.arange(E), 0))
        p_of = csort % P
        t_of = csort // P
        worder = np.argsort(c, kind="stable")
        wdeg_all[i, p_of, t_of * S + crank] = w[worder].astype(np.float16)
        for k in range(CORES):
            sel = (t_of >= k * TPC) & (t_of < (k + 1) * TPC)
            wdeg_own[k, i, p_of[sel], (t_of[sel] - k * TPC) * S + crank[sel]] = \
                w[worder][sel].astype(np.float16)

    K_lo = max(K_lo, 1)
    K_hi = max(K_hi, 1) if cfg.HI_T > 0 else 0
    cfg.K_lo, cfg.K_hi, cfg.K_u = K_lo, K_hi, K_lo + K_hi

    # --- per-core metadata arrays ---
    CH = TPC * cfg.K_u                     # chunks per core per set
    colloc = np.zeros((CORES, A, P, CH), np.float32)
    wchunk = np.zeros((CORES, A, P, CH), np.float32)
    gidx_lo = np.zeros((CORES, A, 16, TPC * K_lo * 8), np.int16)
    gidx_hi = np.zeros((CORES, A, 16, max(TPC * K_hi * 8, 1)), np.int16)

    for i in range(A):
        c_s, r_s, w_s, t_s, hi_s, rank = per_set[i]
        core = t_s // TPC
        tloc = t_s % TPC
        kk = rank // P          # chunk index within (tile, half)
        jj = rank % P           # partition
        lo_m = hi_s == 0
        col_idx = np.where(lo_m, tloc * cfg.K_u + kk, tloc * cfg.K_u + K_lo + kk)
        colloc[core, i, jj, col_idx] = (c_s % P).astype(np.float32)
        wchunk[core, i, jj, col_idx] = w_s.astype(np.float32)
        gi = np.where(lo_m, r_s, r_s - cfg.LO_ROWS).astype(np.int16)
        pos = tloc * (np.where(lo_m, K_lo, K_hi) * P) + rank
        lo_sel = lo_m
        gidx_lo[core[lo_sel], i, pos[lo_sel] % 16, pos[lo_sel] // 16] = gi[lo_sel]
        if cfg.HI_T > 0:
            hi_sel = ~lo_m
            gidx_hi[core[hi_sel], i, pos[hi_sel] % 16, pos[hi_sel] // 16] = gi[hi_sel]

    # --- dense-phase inputs ---
    xpad = np.zeros((cfg.N2, P), np.float32)
    xpad[:N] = x
    xT_all = np.ascontiguousarray(xpad.T).astype(np.float16)        # [P, N2]
    WT = np.ascontiguousarray(conv_w.transpose(0, 2, 1)).astype(np.float16)  # [A,P,P]
    linWT = np.ascontiguousarray(lin_w.T).astype(np.float16)        # [P,P]
    lin_b_row = lin_b.reshape(1, P).astype(np.float16)
    iota_row = np.tile(np.arange(P, dtype=np.float16), (P, 1))       # [P,P]
    ones_row = np.ones((1, P), np.float16)
    pidx_col = np.arange(P, dtype=np.float32).reshape(P, 1)           # [P,1]
    b_bcast = np.tile(conv_b.reshape(A, 1, P), (1, P, 1)).astype(np.float16)  # [A,P,P]

    in_maps = []
    for k in range(CORES):
        m = dict(
            xT_all=xT_all,
            xT_own=np.ascontiguousarray(xT_all[:, k * cfg.OWN:(k + 1) * cfg.OWN]),
            wdeg_all=wdeg_all,
            wdeg_own=wdeg_own[k],
            WT=WT, linWT=linWT, lin_b_row=lin_b_row,
            iota_row=iota_row, ones_row=ones_row,
            pidx_col=pidx_col, b_bcast=b_bcast,
            colloc=colloc[k], wchunk=wchunk[k],
            gidx_lo=np.tile(gidx_lo[k], (1, 8, 1)),
        )
        if cfg.HI_T > 0:
            m["gidx_hi"] = np.tile(gidx_hi[k], (1, 8, 1))
        in_maps.append(m)
    return in_maps


def _build(cfg):
    """Build the single SPMD Bass program."""
    nc = bacc.Bacc(num_swdge_queues=NQ)
    A, TPC, NT, S = cfg.A, cfg.TPC, cfg.NT, cfg.S
    K_lo, K_hi, K_u = cfg.K_lo, cfg.K_hi, cfg.K_u
    CH = TPC * K_u
    Alu = mybir.AluOpType
    Act = mybir.ActivationFunctionType

    # ---- I/O ----
    xT_all = nc.dram_tensor("xT_all", [P, cfg.N2], f16, kind="ExternalInput")
    xT_own = nc.dram_tensor("xT_own", [P, cfg.OWN], f16, kind="ExternalInput")
    wdeg_all = nc.dram_tensor("wdeg_all", [A, P, NT * S], f16, kind="ExternalInput")
    wdeg_own = nc.dram_tensor("wdeg_own", [A, P, TPC * S], f16, kind="ExternalInput")
    WT = nc.dram_tensor("WT", [A, P, P], f16, kind="ExternalInput")
    linWT = nc.dram_tensor("linWT", [P, P], f16, kind="ExternalInput")
    lin_b_row = nc.dram_tensor("lin_b_row", [1, P], f16, kind="ExternalInput")
    iota_row = nc.dram_tensor("iota_row", [P, P], f16, kind="ExternalInput")
    ones_row = nc.dram_tensor("ones_row", [1, P], f16, kind="ExternalInput")
    pidx_col = nc.dram_tensor("pidx_col", [P, 1], f32, kind="ExternalInput")
    b_bcast = nc.dram_tensor("b_bcast", [A, P, P], f16, kind="ExternalInput")
    colloc_d = nc.dram_tensor("colloc", [A, P, CH], f32, kind="ExternalInput")
    wchunk_d = nc.dram_tensor("wchunk", [A, P, CH], f32, kind="ExternalInput")
    gidx_lo_d = nc.dram_tensor("gidx_lo", [A, 128, TPC * K_lo * 8], i16,
                               kind="ExternalInput")
    gidx_hi_d = (nc.dram_tensor("gidx_hi", [A, 128, TPC * K_hi * 8], i16,
                                kind="ExternalInput") if cfg.HI_T > 0 else None)

    hs0 = nc.dram_tensor("hs0", [cfg.OWN, P], f32, kind="ExternalOutput")
    outs = [nc.dram_tensor(f"out{i}", [cfg.OWN, P], f32, kind="ExternalOutput")
            for i in range(A)]

    h_lo = [nc.dram_tensor(f"h{i}_lo", [cfg.LO_ROWS, P], f16) for i in range(A)]
    h_hi = [nc.dram_tensor(f"h{i}_hi", [max(cfg.HI_ROWS, P), P], f16)
            for i in range(A)]

    qctr = [0]  # round-robin SWDGE queue counter

    with tile.TileContext(nc) as tc:
        with (
            tc.tile_pool(name="const", bufs=1) as cpool,
            tc.tile_pool(name="meta", bufs=1) as mpool,
            tc.tile_pool(name="degio", bufs=2) as dpool,
            tc.tile_pool(name="degres", bufs=1) as rpool,
            tc.tile_pool(name="xw", bufs=3) as xpool,
            tc.tile_pool(name="hstage", bufs=3) as hpool,
            tc.tile_pool(name="glo", bufs=2) as glo_pool,
            tc.tile_pool(name="ghi", bufs=2) as ghi_pool,
            tc.tile_pool(name="gix", bufs=2) as gix_pool,
            tc.tile_pool(name="bw", bufs=6) as bwpool,
            tc.tile_pool(name="outst", bufs=3) as opool,
            tc.tile_pool(name="psd", bufs=4, space="PSUM") as psd,
            tc.tile_pool(name="pss", bufs=4, space="PSUM") as pss,
        ):
            # ---- constants to SBUF ----
            iota_t = cpool.tile([P, P], f16)
            nc.sync.dma_start(out=iota_t[:], in_=iota_row[:])
            ones_t = cpool.tile([1, P], f16)
            nc.sync.dma_start(out=ones_t[:], in_=ones_row[:])
            linb_t = cpool.tile([1, P], f16)
            nc.sync.dma_start(out=linb_t[:], in_=lin_b_row[:])
            linWT_t = cpool.tile([P, P], f16)
            nc.sync.dma_start(out=linWT_t[:], in_=linWT[:])
            pidx_t = cpool.tile([P, 1], f32)
            nc.sync.dma_start(out=pidx_t[:], in_=pidx_col[:])
            WT_t = []
            b_t = []
            for i in range(A):
                wt = cpool.tile([P, P], f16, tag=f"WT{i}")
                nc.sync.dma_start(out=wt[:], in_=WT[i, :, :])
                WT_t.append(wt)
                bt = cpool.tile([P, P], f16, tag=f"bt{i}")
                nc.sync.dma_start(out=bt[:], in_=b_bcast[i, :, :])
                b_t.append(bt)
            colloc_t = []
            wchunk_t = []
            for i in range(A):
                ct = mpool.tile([P, CH], f32, tag=f"colloc{i}")
                nc.sync.dma_start(out=ct[:], in_=colloc_d[i, :, :])
                colloc_t.append(ct)
                wt = mpool.tile([P, CH], f32, tag=f"wchunk{i}")
                nc.sync.dma_start(out=wt[:], in_=wchunk_d[i, :, :])
                wchunk_t.append(wt)

            # ---- degree phase ----
            def deg_pipeline(src, n_tiles, tag):
                deg = rpool.tile([P, n_tiles], f32, tag=f"deg_{tag}")
                GRP = max(1, min(n_tiles, 4096 // S))
                for g0 in range(0, n_tiles, GRP):
                    g1 = min(n_tiles, g0 + GRP)
                    wt = dpool.tile([P, GRP * S], f16, tag="degload")
                    nc.sync.dma_start(out=wt[:, :(g1 - g0) * S],
                                      in_=src[:, g0 * S:g1 * S])
                    nc.vector.tensor_reduce(
                        out=deg[:, g0:g1],
                        in_=wt[:, :(g1 - g0) * S].rearrange("p (g s) -> p g s", s=S),
                        axis=mybir.AxisListType.X, op=Alu.add)
                m = rpool.tile([P, n_tiles], f32, tag=f"m_{tag}")
                nc.vector.tensor_scalar(out=m[:], in0=deg[:], scalar1=0.0,
                                        scalar2=None, op0=Alu.is_gt)
                degsafe = rpool.tile([P, n_tiles], f32, tag=f"ds_{tag}")
                nc.vector.tensor_scalar(out=degsafe[:], in0=deg[:], scalar1=1.0,
                                        scalar2=None, op0=Alu.add)
                nc.vector.tensor_tensor(out=degsafe[:], in0=degsafe[:], in1=m[:],
                                        op=Alu.subtract)
                s = rpool.tile([P, n_tiles], f32, tag=f"s_{tag}")
                nc.scalar.activation(out=s[:], in_=degsafe[:], func=Act.Sqrt)
                dinv_safe = rpool.tile([P, n_tiles], f32, tag=f"dvs_{tag}")
                nc.vector.reciprocal(out=dinv_safe[:], in_=s[:])
                dinv_m = rpool.tile([P, n_tiles], f32, tag=f"dvm_{tag}")
                nc.vector.tensor_tensor(out=dinv_m[:], in0=dinv_safe[:], in1=m[:],
                                        op=Alu.mult)
                return m, dinv_m, dinv_safe, s

            dinv_row = []   # [P, NT] f32 masked, per set
            dinv_col = []   # [P, TPC] f32 safe, per set
            s_col = []      # [P, TPC] f32 safe sqrt(deg), per set
            for i in range(A):
                _, dm, _, _ = deg_pipeline(wdeg_all[i, :, :], NT, f"all{i}")
                dinv_row.append(dm)
                _, _, dvs, s_own = deg_pipeline(wdeg_own[i, :, :], TPC, f"own{i}")
                dinv_col.append(dvs)
                s_col.append(s_own)

            # ---- dense phase for one set: h'_i over all NT tiles, batched ----
            def dense_phase(i):
                for t0 in range(0, NT, TB):
                    tb = min(TB, NT - t0)
                    xt = xpool.tile([P, TB * P], f16, tag="xt")
                    nc.sync.dma_start(out=xt[:, :tb * P],
                                      in_=xT_all[:, t0 * P:(t0 + tb) * P])
                    hst = hpool.tile([P, TB * P], f16, tag="hst")
                    for k in range(tb):
                        t = t0 + k
                        ps = psd.tile([P, P], f32, tag="psd")
                        nc.tensor.matmul(out=ps[:], lhsT=xt[:, k * P:(k + 1) * P],
                                         rhs=WT_t[i][:], start=True, stop=True)
                        nc.scalar.activation(out=hst[:, k * P:(k + 1) * P],
                                             in_=ps[:], func=Act.Copy,
                                             scale=dinv_row[i][:, t:t + 1])
                    # batched write: SBUF [p, (k f)] -> DRAM rows t0*P..(t0+tb)*P
                    if t0 + tb <= cfg.LO_T:
                        dst = h_lo[i][t0 * P:(t0 + tb) * P, :]
                    else:
                        assert t0 >= cfg.LO_T, "TB must divide LO_T"
                        tt = t0 - cfg.LO_T
                        dst = h_hi[i][tt * P:(tt + tb) * P, :]
                    nc.scalar.dma_start(
                        out=dst.rearrange("(k p) f -> p k f", p=P),
                        in_=hst[:, :tb * P].rearrange("p (k f) -> p k f", f=P))

            # ---- hs0 (own rows only), batched ----
            def lin_phase():
                for t0 in range(0, TPC, TB):
                    tb = min(TB, TPC - t0)
                    xo = xpool.tile([P, TB * P], f16, tag="xo")
                    nc.sync.dma_start(out=xo[:, :tb * P],
                                      in_=xT_own[:, t0 * P:(t0 + tb) * P])
                    ost = opool.tile([P, TB * P], f32, tag="ost")
                    for k in range(tb):
                        ps = psd.tile([P, P], f32, tag="psd")
                        nc.tensor.matmul(out=ps[:], lhsT=xo[:, k * P:(k + 1) * P],
                                         rhs=linWT_t[:], start=True, stop=False)
                        nc.tensor.matmul(out=ps[:], lhsT=ones_t[:], rhs=linb_t[:],
                                         start=False, stop=True)
                        nc.scalar.activation(out=ost[:, k * P:(k + 1) * P],
                                             in_=ps[:], func=Act.Relu)
                    nc.sync.dma_start(
                        out=hs0[t0 * P:(t0 + tb) * P, :].rearrange(
                            "(k p) f -> p k f", p=P),
                        in_=ost[:, :tb * P].rearrange("p (k f) -> p k f", f=P))

            # ---- edge phase for one set ----
            def edge_phase(i):
                # whole-set gather index loads (2 DMAs)
                gixlo = gix_pool.tile([128, TPC * K_lo * 8], i16, tag="gixlo")
                nc.sync.dma_start(out=gixlo[:], in_=gidx_lo_d[i, :, :])
                if cfg.HI_T > 0:
                    gixhi = gix_pool.tile([128, TPC * K_hi * 8], i16, tag="gixhi")
                    nc.sync.dma_start(out=gixhi[:], in_=gidx_hi_d[i, :, :])

                ost = None
                ost_t0 = 0
                for g in range(cfg.n_groups):
                    tg = cfg.group_tiles[g]
                    t0 = g * min(GATHER_GROUP, TPC)
                    GT = min(GATHER_GROUP, TPC)
                    # lo gather
                    glo = glo_pool.tile([P, GT * K_lo, P], f16, tag="glo")
                    L = tg * K_lo * P
                    off = t0 * K_lo * 8
                    nc.gpsimd.dma_gather(
                        out_ap=glo[:, :tg * K_lo, :], in_ap=h_lo[i][:, :],
                        idxs_ap=gixlo[:, off:off + L // 16],
                        num_idxs=L, num_idxs_reg=L, elem_size=P,
                        single_packet=False, queue_num=qctr[0] % NQ)
                    qctr[0] += 1
                    # hi gather
                    if cfg.HI_T > 0:
                        ghi = ghi_pool.tile([P, GT * K_hi, P], f16, tag="ghi")
                        Lh = tg * K_hi * P
                        offh = t0 * K_hi * 8
                        nc.gpsimd.dma_gather(
                            out_ap=ghi[:, :tg * K_hi, :], in_ap=h_hi[i][:, :],
                            idxs_ap=gixhi[:, offh:offh + Lh // 16],
                            num_idxs=Lh, num_idxs_reg=Lh, elem_size=P,
                            single_packet=False, queue_num=qctr[0] % NQ)
                        qctr[0] += 1
                    for tl in range(tg):
                        tt = t0 + tl
                        if ost is None:
                            ost = opool.tile([P, TB * P], f32, tag="ost")
                            ost_t0 = tt
                        ps = pss.tile([P, P], f32, tag="pss")
                        for k in range(K_u):
                            gc = tt * K_u + k
                            bw = bwpool.tile([P, P], f16, tag="bw")
                            nc.vector.tensor_scalar(
                                out=bw[:], in0=iota_t[:],
                                scalar1=colloc_t[i][:, gc:gc + 1],
                                scalar2=wchunk_t[i][:, gc:gc + 1],
                                op0=Alu.is_equal, op1=Alu.mult)
                            if k < K_lo:
                                src = glo[:, tl * K_lo + k, :]
                            else:
                                src = ghi[:, tl * K_hi + (k - K_lo), :]
                            nc.tensor.matmul(out=ps[:], lhsT=bw[:], rhs=src,
                                             start=(k == 0), stop=False)
                        # bias: += diag(s2) @ b_bcast  ==> (t + s2*b)
                        dg = bwpool.tile([P, P], f16, tag="dg")
                        nc.vector.tensor_scalar(
                            out=dg[:], in0=iota_t[:],
                            scalar1=pidx_t[:, 0:1],
                            scalar2=s_col[i][:, tt:tt + 1],
                            op0=Alu.is_equal, op1=Alu.mult)
                        nc.tensor.matmul(out=ps[:], lhsT=dg[:], rhs=b_t[i][:],
                                         start=False, stop=True)
                        ko = tt - ost_t0
                        nc.vector.tensor_scalar(
                            out=ost[:, ko * P:(ko + 1) * P], in0=ps[:],
                            scalar1=dinv_col[i][:, tt:tt + 1],
                            scalar2=0.0, op0=Alu.mult, op1=Alu.max)
                        if ko == TB - 1 or tt == TPC - 1:
                            tb = ko + 1
                            nc.sync.dma_start(
                                out=outs[i][ost_t0 * P:(ost_t0 + tb) * P, :]
                                .rearrange("(k p) f -> p k f", p=P),
                                in_=ost[:, :tb * P].rearrange(
                                    "p (k f) -> p k f", f=P))
                            ost = None

            # ---- pipelined emission ----
            dense_phase(0)
            lin_phase()
            dense_phase(1)
            edge_phase(0)
            dense_phase(2)
            edge_phase(1)
            edge_phase(2)
    nc.finalize()
    return nc


def _assemble(cfg, results):
    """results: list of per-core output dicts -> full outputs tuple."""
    N, A = cfg.N, cfg.A
    hs = []
    h0 = np.concatenate([results[k]["hs0"] for k in range(CORES)], axis=0)[:N]
    hs.append(h0.astype(np.float32))
    for i in range(A):
        o = np.concatenate([results[k][f"out{i}"] for k in range(CORES)], axis=0)[:N]
        hs.append(o.astype(np.float32))
    return tuple(hs)


def kernel(x, edge_index, edge_attr, lin_w, lin_b, conv_w, conv_b):
    global LAST_RESULTS
    x = np.asarray(x, np.float32)
    edge_index = np.asarray(edge_index)
    edge_attr = np.asarray(edge_attr, np.float32)
    lin_w = np.asarray(lin_w, np.float32)
    lin_b = np.asarray(lin_b, np.float32)
    conv_w = np.asarray(conv_w, np.float32)
    conv_b = np.asarray(conv_b, np.float32)

    N, D = x.shape
    A, _, E = edge_index.shape
    assert D == P
    cfg = _make_cfg(N, E, A)
    in_maps = _prep(cfg, x, edge_index, edge_attr, lin_w, lin_b, conv_w, conv_b)
    nc = _build(cfg)

    from concourse.bass_utils import run_bass_kernel_spmd
    res = run_bass_kernel_spmd(nc, in_maps, list(range(CORES)), trace=TRACE)
    LAST_RESULTS = res
    return _assemble(cfg, res.results)


# ---------- simulation path (for testing on small configs) ----------

def run_sim(x, edge_index, edge_attr, lin_w, lin_b, conv_w, conv_b,
            cores=None):
    """Run each core through CoreSim; returns assembled outputs."""
    from concourse import bass_interp
    x = np.asarray(x, np.float32)
    edge_index = np.asarray(edge_index)
    edge_attr = np.asarray(edge_attr, np.float32)
    N, D = x.shape
    A, _, E = edge_index.shape
    cfg = _make_cfg(N, E, A)
    in_maps = _prep(cfg, x, edge_index, edge_attr,
                    np.asarray(lin_w, np.float32), np.asarray(lin_b, np.float32),
                    np.asarray(conv_w, np.float32), np.asarray(conv_b, np.float32))
    results = []
    for k in (range(CORES) if cores is None else cores):
        nc = _build(cfg)
        sim = bass_interp.CoreSim(nc, core_id=0)
        sim.assign_tensors(in_maps[k])
        sim.simulate()
        results.append({name: sim.tensor(name).copy()
                        for name in ["hs0"] + [f"out{i}" for i in range(A)]})
    if cores is not None:
        return cfg, results
    return _assemble(cfg, results)


# revision 8
# speedup vs baseline: 1.8904x; 1.3790x over previous
"""GCN cascade layer (3 parallel GCNConv + 1 linear head) on 8 Trainium2 cores.

Math (per edge set i):
    deg[c]   = sum_{e: col=c} w[e]
    dinv     = deg>0 ? 1/sqrt(deg) : 0
    h        = x @ W_i.T
    out[c]   = relu( sum_e dinv[row]*w*dinv[c] * h[row] + b )
Reformulated so per-edge work is gather + weighted scatter only:
    h'[r]    = h[r] * dinv_m[r]            (dinv_m masked: 0 where deg==0)
    t[c]     = sum_{e->c} w[e] * h'[row[e]]
    out[c]   = relu( dinv_s[c] * (t[c] + s2[c]*b) )
  where dinv_s = 1/sqrt(max(deg,1-ish)) (==1 at deg==0), s2 = 1/dinv_s,
  so dinv_s*s2 == 1 and deg==0 columns give relu(b) exactly like the ref.

Distribution: output columns (nodes) sharded over 8 cores in contiguous
128-col tiles; h' computed replicated on every core (one pass over x per
set); per-core edges bucketed by 128-col tile on the host; gather of
h'[row] via the custom SWDGE dma_gather; scatter = one-hot-weighted matmul
accumulating into PSUM (exact segment sum on the TensorEngine).

Perf structure (v2): dma_gather descriptor generation runs on Q7 core pair
[2q, 2q+1] selected by queue_num — with num_swdge_queues=4 and round-robin
queue assignment, four gathers generate descriptors concurrently. Emission
is pipelined (dense_0, dense_1, edge_0, dense_2, edge_1, edge_2) so the PE
has dense work queued while the first gathers run, and DMAs are batched
4 tiles per transfer to cut sync-engine issue overhead.
"""

import sys

sys.path.insert(0, "/opt/trn_rl_repo")

import math
from dataclasses import dataclass, field

import numpy as np

import concourse.bass as bass
import concourse.bacc as bacc
import concourse.mybir as mybir
from concourse import tile, library_config

P = 128          # partitions / feature dim
CORES = 8
GATHER_GROUP = 7  # node tiles per dma_gather call group
NQ = 4            # SWDGE queues (gather desc-gen concurrency)
TB = 4            # tiles per batched DMA

f16 = mybir.dt.float16
f32 = mybir.dt.float32
i16 = mybir.dt.int16

# set by run(): BassKernelResults of the last hardware run (for profiling)
LAST_RESULTS = None
TRACE = False


@dataclass
class Cfg:
    N: int
    E: int
    A: int
    TPC: int          # node tiles per core
    NT: int           # total node tiles (CORES*TPC)
    N2: int           # padded node count (NT*P)
    OWN: int          # cols/rows owned per core (TPC*P)
    LO_T: int         # tiles in the "lo" half of h'
    LO_ROWS: int
    HI_T: int
    HI_ROWS: int
    S: int            # padded max col-degree slots
    K_lo: int         # lo chunks per node tile
    K_hi: int
    K_u: int          # K_lo + K_hi
    n_groups: int = 0
    group_tiles: list = field(default_factory=list)


def _make_cfg(N, E, A):
    TPC = math.ceil(N / (CORES * P))
    NT = CORES * TPC
    N2 = NT * P
    LO_T = (NT + 1) // 2
    LO_ROWS = LO_T * P
    HI_T = NT - LO_T
    HI_ROWS = HI_T * P
    assert LO_ROWS < 32768 and HI_ROWS < 32768, "int16 gather index overflow"
    cfg = Cfg(N=N, E=E, A=A, TPC=TPC, NT=NT, N2=N2, OWN=TPC * P,
              LO_T=LO_T, LO_ROWS=LO_ROWS, HI_T=HI_T, HI_ROWS=HI_ROWS,
              S=0, K_lo=0, K_hi=0, K_u=0)
    g = min(GATHER_GROUP, TPC)
    cfg.n_groups = math.ceil(TPC / g)
    cfg.group_tiles = [min(g, TPC - i * g) for i in range(cfg.n_groups)]
    return cfg


def _prep(cfg, x, edge_index, edge_attr, lin_w, lin_b, conv_w, conv_b):
    """Host-side sharding/layout prep. Returns the per-core input dict list."""
    A, N, E = cfg.A, cfg.N, cfg.E
    TPC, NT = cfg.TPC, cfg.NT

    r_all = edge_index[:, 0, :].astype(np.int64)   # [A,E]
    c_all = edge_index[:, 1, :].astype(np.int64)
    w_all = edge_attr.astype(np.float32)

    # --- degree (host only for LAYOUT: slot ranks + max degree) ---
    deg_counts = np.zeros((A, cfg.N2), np.int64)
    for i in range(A):
        deg_counts[i] = np.bincount(c_all[i], minlength=cfg.N2)
    S = int(deg_counts.max())
    S = max(S, 1)
    cfg.S = S

    # wdeg_all[i, p, tglob*S + s] = s-th weight of col (tglob*P + p)
    wdeg_all = np.zeros((A, P, NT * S), np.float16)
    wdeg_own = np.zeros((CORES, A, P, TPC * S), np.float16)

    # --- edge bucketing ---
    K_lo = K_hi = 0
    per_set = []
    for i in range(A):
        c = c_all[i]
        r = r_all[i]
        w = w_all[i]
        tile_of = c // P
        is_hi = (r >= cfg.LO_ROWS).astype(np.int64)
        order = np.lexsort((is_hi, tile_of))
        c_s, r_s, w_s, t_s, hi_s = c[order], r[order], w[order], tile_of[order], is_hi[order]
        seg_key = t_s * 2 + hi_s
        seg_change = np.empty(E, np.bool_)
        seg_change[0] = True
        seg_change[1:] = seg_key[1:] != seg_key[:-1]
        seg_start_idx = np.flatnonzero(seg_change)
        starts = np.zeros(E, np.int64)
        starts[seg_start_idx] = seg_start_idx
        starts = np.maximum.accumulate(starts)
        rank = np.arange(E) - starts
        n_lo = np.bincount(t_s[hi_s == 0], minlength=NT)
        n_hi = np.bincount(t_s[hi_s == 1], minlength=NT)
        K_lo = max(K_lo, int(math.ceil(n_lo.max() / P)))
        K_hi = max(K_hi, int(math.ceil(n_hi.max() / P)))
        per_set.append((c_s, r_s, w_s, t_s, hi_s, rank))

        # degree slot layout
        csort = np.sort(c)
        crank = np.arange(E) - np.maximum.accumulate(
            np.where(np.r_[True, csort[1:] != csort[:-1]], np.arange(E), 0))
        p_of = csort % P
        t_of = csort // P
        worder = np.argsort(c, kind="stable")
        wdeg_all[i, p_of, t_of * S + crank] = w[worder].astype(np.float16)
        for k in range(CORES):
            sel = (t_of >= k * TPC) & (t_of < (k + 1) * TPC)
            wdeg_own[k, i, p_of[sel], (t_of[sel] - k * TPC) * S + crank[sel]] = \
                w[worder][sel].astype(np.float16)

    K_lo = max(K_lo, 1)
    K_hi = max(K_hi, 1) if cfg.HI_T > 0 else 0
    cfg.K_lo, cfg.K_hi, cfg.K_u = K_lo, K_hi, K_lo + K_hi

    # --- per-core metadata arrays ---
    CH = TPC * cfg.K_u                     # chunks per core per set
    # bw[core, i, jj, chunk*P + c] = one-hot(col%P == c) * w, prebuilt on host
    bwall = np.zeros((CORES, A, P, CH * P), np.float16)
    gidx_lo = np.zeros((CORES, A, 16, TPC * K_lo * 8), np.int16)
    gidx_hi = np.zeros((CORES, A, 16, max(TPC * K_hi * 8, 1)), np.int16)

    for i in range(A):
        c_s, r_s, w_s, t_s, hi_s, rank = per_set[i]
        core = t_s // TPC
        tloc = t_s % TPC
        kk = rank // P          # chunk index within (tile, half)
        jj = rank % P           # partition
        lo_m = hi_s == 0
        col_idx = np.where(lo_m, tloc * cfg.K_u + kk, tloc * cfg.K_u + K_lo + kk)
        bwall[core, i, jj, col_idx * P + (c_s % P)] = w_s.astype(np.float16)
        gi = np.where(lo_m, r_s, r_s - cfg.LO_ROWS).astype(np.int16)
        pos = tloc * (np.where(lo_m, K_lo, K_hi) * P) + rank
        lo_sel = lo_m
        gidx_lo[core[lo_sel], i, pos[lo_sel] % 16, pos[lo_sel] // 16] = gi[lo_sel]
        if cfg.HI_T > 0:
            hi_sel = ~lo_m
            gidx_hi[core[hi_sel], i, pos[hi_sel] % 16, pos[hi_sel] // 16] = gi[hi_sel]

    # --- dense-phase inputs ---
    xpad = np.zeros((cfg.N2, P), np.float32)
    xpad[:N] = x
    xT_all = np.ascontiguousarray(xpad.T).astype(np.float16)        # [P, N2]
    WT = np.ascontiguousarray(conv_w.transpose(0, 2, 1)).astype(np.float16)  # [A,P,P]
    linWT = np.ascontiguousarray(lin_w.T).astype(np.float16)        # [P,P]
    lin_b_row = lin_b.reshape(1, P).astype(np.float16)
    ident = np.eye(P, dtype=np.float16)                              # [P,P]
    ones_row = np.ones((1, P), np.float16)
    b_bcast = np.tile(conv_b.reshape(A, 1, P), (1, P, 1)).astype(np.float16)  # [A,P,P]

    in_maps = []
    for k in range(CORES):
        m = dict(
            xT_all=xT_all,
            xT_own=np.ascontiguousarray(xT_all[:, k * cfg.OWN:(k + 1) * cfg.OWN]),
            wdeg_all=wdeg_all,
            wdeg_own=wdeg_own[k],
            WT=WT, linWT=linWT, lin_b_row=lin_b_row,
            ident=ident, ones_row=ones_row, b_bcast=b_bcast,
            bwall=bwall[k],
            gidx_lo=np.tile(gidx_lo[k], (1, 8, 1)),
        )
        if cfg.HI_T > 0:
            m["gidx_hi"] = np.tile(gidx_hi[k], (1, 8, 1))
        in_maps.append(m)
    return in_maps


def _build(cfg):
    """Build the single SPMD Bass program."""
    nc = bacc.Bacc(num_swdge_queues=NQ)
    A, TPC, NT, S = cfg.A, cfg.TPC, cfg.NT, cfg.S
    K_lo, K_hi, K_u = cfg.K_lo, cfg.K_hi, cfg.K_u
    CH = TPC * K_u
    Alu = mybir.AluOpType
    Act = mybir.ActivationFunctionType

    # ---- I/O ----
    xT_all = nc.dram_tensor("xT_all", [P, cfg.N2], f16, kind="ExternalInput")
    xT_own = nc.dram_tensor("xT_own", [P, cfg.OWN], f16, kind="ExternalInput")
    wdeg_all = nc.dram_tensor("wdeg_all", [A, P, NT * S], f16, kind="ExternalInput")
    wdeg_own = nc.dram_tensor("wdeg_own", [A, P, TPC * S], f16, kind="ExternalInput")
    WT = nc.dram_tensor("WT", [A, P, P], f16, kind="ExternalInput")
    linWT = nc.dram_tensor("linWT", [P, P], f16, kind="ExternalInput")
    lin_b_row = nc.dram_tensor("lin_b_row", [1, P], f16, kind="ExternalInput")
    ident_d = nc.dram_tensor("ident", [P, P], f16, kind="ExternalInput")
    ones_row = nc.dram_tensor("ones_row", [1, P], f16, kind="ExternalInput")
    b_bcast = nc.dram_tensor("b_bcast", [A, P, P], f16, kind="ExternalInput")
    bwall_d = nc.dram_tensor("bwall", [A, P, CH * P], f16, kind="ExternalInput")
    gidx_lo_d = nc.dram_tensor("gidx_lo", [A, 128, TPC * K_lo * 8], i16,
                               kind="ExternalInput")
    gidx_hi_d = (nc.dram_tensor("gidx_hi", [A, 128, TPC * K_hi * 8], i16,
                                kind="ExternalInput") if cfg.HI_T > 0 else None)

    hs0 = nc.dram_tensor("hs0", [cfg.OWN, P], f32, kind="ExternalOutput")
    outs = [nc.dram_tensor(f"out{i}", [cfg.OWN, P], f32, kind="ExternalOutput")
            for i in range(A)]

    h_lo = [nc.dram_tensor(f"h{i}_lo", [cfg.LO_ROWS, P], f16) for i in range(A)]
    h_hi = [nc.dram_tensor(f"h{i}_hi", [max(cfg.HI_ROWS, P), P], f16)
            for i in range(A)]

    qctr = [0]  # round-robin SWDGE queue counter

    with tile.TileContext(nc) as tc:
        with (
            tc.tile_pool(name="const", bufs=1) as cpool,
            tc.tile_pool(name="meta", bufs=1) as mpool,
            tc.tile_pool(name="degio", bufs=2) as dpool,
            tc.tile_pool(name="degres", bufs=1) as rpool,
            tc.tile_pool(name="xw", bufs=3) as xpool,
            tc.tile_pool(name="hstage", bufs=3) as hpool,
            tc.tile_pool(name="glo", bufs=3) as glo_pool,
            tc.tile_pool(name="ghi", bufs=3) as ghi_pool,
            tc.tile_pool(name="gix", bufs=2) as gix_pool,
            tc.tile_pool(name="bw", bufs=3) as bwpool,
            tc.tile_pool(name="dg", bufs=4) as dgpool,
            tc.tile_pool(name="outst", bufs=3) as opool,
            tc.tile_pool(name="psd", bufs=4, space="PSUM") as psd,
            tc.tile_pool(name="pss", bufs=4, space="PSUM") as pss,
        ):
            # ---- constants to SBUF ----
            ident_t = cpool.tile([P, P], f16)
            nc.sync.dma_start(out=ident_t[:], in_=ident_d[:])
            ones_t = cpool.tile([1, P], f16)
            nc.sync.dma_start(out=ones_t[:], in_=ones_row[:])
            linb_t = cpool.tile([1, P], f16)
            nc.sync.dma_start(out=linb_t[:], in_=lin_b_row[:])
            linWT_t = cpool.tile([P, P], f16)
            nc.sync.dma_start(out=linWT_t[:], in_=linWT[:])
            WT_t = []
            b_t = []
            for i in range(A):
                wt = cpool.tile([P, P], f16, tag=f"WT{i}")
                nc.sync.dma_start(out=wt[:], in_=WT[i, :, :])
                WT_t.append(wt)
                bt = cpool.tile([P, P], f16, tag=f"bt{i}")
                nc.sync.dma_start(out=bt[:], in_=b_bcast[i, :, :])
                b_t.append(bt)

            # ---- degree phase ----
            def deg_pipeline(src, n_tiles, tag):
                deg = rpool.tile([P, n_tiles], f32, tag=f"deg_{tag}")
                GRP = max(1, min(n_tiles, 4096 // S))
                for g0 in range(0, n_tiles, GRP):
                    g1 = min(n_tiles, g0 + GRP)
                    wt = dpool.tile([P, GRP * S], f16, tag="degload")
                    nc.sync.dma_start(out=wt[:, :(g1 - g0) * S],
                                      in_=src[:, g0 * S:g1 * S])
                    nc.vector.tensor_reduce(
                        out=deg[:, g0:g1],
                        in_=wt[:, :(g1 - g0) * S].rearrange("p (g s) -> p g s", s=S),
                        axis=mybir.AxisListType.X, op=Alu.add)
                m = rpool.tile([P, n_tiles], f32, tag=f"m_{tag}")
                nc.vector.tensor_scalar(out=m[:], in0=deg[:], scalar1=0.0,
                                        scalar2=None, op0=Alu.is_gt)
                degsafe = rpool.tile([P, n_tiles], f32, tag=f"ds_{tag}")
                nc.vector.tensor_scalar(out=degsafe[:], in0=deg[:], scalar1=1.0,
                                        scalar2=None, op0=Alu.add)
                nc.vector.tensor_tensor(out=degsafe[:], in0=degsafe[:], in1=m[:],
                                        op=Alu.subtract)
                s = rpool.tile([P, n_tiles], f32, tag=f"s_{tag}")
                nc.scalar.activation(out=s[:], in_=degsafe[:], func=Act.Sqrt)
                dinv_safe = rpool.tile([P, n_tiles], f32, tag=f"dvs_{tag}")
                nc.vector.reciprocal(out=dinv_safe[:], in_=s[:])
                dinv_m = rpool.tile([P, n_tiles], f32, tag=f"dvm_{tag}")
                nc.vector.tensor_tensor(out=dinv_m[:], in0=dinv_safe[:], in1=m[:],
                                        op=Alu.mult)
                return m, dinv_m, dinv_safe, s

            dinv_row = []   # [P, NT] f32 masked, per set
            dinv_col = []   # [P, TPC] f32 safe, per set
            s_col = []      # [P, TPC] f32 safe sqrt(deg), per set
            for i in range(A):
                _, dm, _, _ = deg_pipeline(wdeg_all[i, :, :], NT, f"all{i}")
                dinv_row.append(dm)
                _, _, dvs, s_own = deg_pipeline(wdeg_own[i, :, :], TPC, f"own{i}")
                dinv_col.append(dvs)
                s_col.append(s_own)

            # ---- dense phase for one set: h'_i over all NT tiles, batched ----
            def dense_phase(i):
                for t0 in range(0, NT, TB):
                    tb = min(TB, NT - t0)
                    xt = xpool.tile([P, TB * P], f16, tag="xt")
                    nc.sync.dma_start(out=xt[:, :tb * P],
                                      in_=xT_all[:, t0 * P:(t0 + tb) * P])
                    hst = hpool.tile([P, TB * P], f16, tag="hst")
                    for k in range(tb):
                        t = t0 + k
                        ps = psd.tile([P, P], f32, tag="psd")
                        nc.tensor.matmul(out=ps[:], lhsT=xt[:, k * P:(k + 1) * P],
                                         rhs=WT_t[i][:], start=True, stop=True)
                        nc.scalar.activation(out=hst[:, k * P:(k + 1) * P],
                                             in_=ps[:], func=Act.Copy,
                                             scale=dinv_row[i][:, t:t + 1])
                    # batched write: SBUF [p, (k f)] -> DRAM rows t0*P..(t0+tb)*P
                    if t0 + tb <= cfg.LO_T:
                        dst = h_lo[i][t0 * P:(t0 + tb) * P, :]
                    else:
                        assert t0 >= cfg.LO_T, "TB must divide LO_T"
                        tt = t0 - cfg.LO_T
                        dst = h_hi[i][tt * P:(tt + tb) * P, :]
                    nc.scalar.dma_start(
                        out=dst.rearrange("(k p) f -> p k f", p=P),
                        in_=hst[:, :tb * P].rearrange("p (k f) -> p k f", f=P))

            # ---- hs0 (own rows only), batched ----
            def lin_phase():
                for t0 in range(0, TPC, TB):
                    tb = min(TB, TPC - t0)
                    xo = xpool.tile([P, TB * P], f16, tag="xo")
                    nc.sync.dma_start(out=xo[:, :tb * P],
                                      in_=xT_own[:, t0 * P:(t0 + tb) * P])
                    ost = opool.tile([P, TB * P], f32, tag="ost")
                    for k in range(tb):
                        ps = psd.tile([P, P], f32, tag="psd")
                        nc.tensor.matmul(out=ps[:], lhsT=xo[:, k * P:(k + 1) * P],
                                         rhs=linWT_t[:], start=True, stop=False)
                        nc.tensor.matmul(out=ps[:], lhsT=ones_t[:], rhs=linb_t[:],
                                         start=False, stop=True)
                        nc.scalar.activation(out=ost[:, k * P:(k + 1) * P],
                                             in_=ps[:], func=Act.Relu)
                    nc.sync.dma_start(
                        out=hs0[t0 * P:(t0 + tb) * P, :].rearrange(
                            "(k p) f -> p k f", p=P),
                        in_=ost[:, :tb * P].rearrange("p (k f) -> p k f", f=P))

            # ---- edge phase for one set ----
            def edge_phase(i):
                # whole-set gather index loads (2 DMAs)
                gixlo = gix_pool.tile([128, TPC * K_lo * 8], i16, tag="gixlo")
                nc.sync.dma_start(out=gixlo[:], in_=gidx_lo_d[i, :, :])
                if cfg.HI_T > 0:
                    gixhi = gix_pool.tile([128, TPC * K_hi * 8], i16, tag="gixhi")
                    nc.sync.dma_start(out=gixhi[:], in_=gidx_hi_d[i, :, :])

                ost = None
                ost_t0 = 0
                for g in range(cfg.n_groups):
                    tg = cfg.group_tiles[g]
                    t0 = g * min(GATHER_GROUP, TPC)
                    GT = min(GATHER_GROUP, TPC)
                    # lo gather
                    glo = glo_pool.tile([P, GT * K_lo, P], f16, tag="glo")
                    L = tg * K_lo * P
                    off = t0 * K_lo * 8
                    nc.gpsimd.dma_gather(
                        out_ap=glo[:, :tg * K_lo, :], in_ap=h_lo[i][:, :],
                        idxs_ap=gixlo[:, off:off + L // 16],
                        num_idxs=L, num_idxs_reg=L, elem_size=P,
                        single_packet=False, queue_num=qctr[0] % NQ)
                    qctr[0] += 1
                    # hi gather
                    if cfg.HI_T > 0:
                        ghi = ghi_pool.tile([P, GT * K_hi, P], f16, tag="ghi")
                        Lh = tg * K_hi * P
                        offh = t0 * K_hi * 8
                        nc.gpsimd.dma_gather(
                            out_ap=ghi[:, :tg * K_hi, :], in_ap=h_hi[i][:, :],
                            idxs_ap=gixhi[:, offh:offh + Lh // 16],
                            num_idxs=Lh, num_idxs_reg=Lh, elem_size=P,
                            single_packet=False, queue_num=qctr[0] % NQ)
                        qctr[0] += 1
                    for tl in range(tg):
                        tt = t0 + tl
                        if ost is None:
                            ost = opool.tile([P, TB * P], f32, tag="ost")
                            ost_t0 = tt
                        # host-prebuilt one-hot*w matrices for this tile
                        bwt = bwpool.tile([P, K_u * P], f16, tag="bw")
                        nc.sync.dma_start(
                            out=bwt[:],
                            in_=bwall_d[i, :, tt * K_u * P:(tt + 1) * K_u * P])
                        ps = pss.tile([P, P], f32, tag="pss")
                        for k in range(K_u):
                            if k < K_lo:
                                src = glo[:, tl * K_lo + k, :]
                            else:
                                src = ghi[:, tl * K_hi + (k - K_lo), :]
                            nc.tensor.matmul(out=ps[:],
                                             lhsT=bwt[:, k * P:(k + 1) * P],
                                             rhs=src,
                                             start=(k == 0), stop=False)
                        # bias: += diag(s2) @ b_bcast  ==> (t + s2*b)
                        dg = dgpool.tile([P, P], f16, tag="dg")
                        nc.vector.tensor_scalar(
                            out=dg[:], in0=ident_t[:],
                            scalar1=s_col[i][:, tt:tt + 1],
                            scalar2=None, op0=Alu.mult)
                        nc.tensor.matmul(out=ps[:], lhsT=dg[:], rhs=b_t[i][:],
                                         start=False, stop=True)
                        ko = tt - ost_t0
                        nc.vector.tensor_scalar(
                            out=ost[:, ko * P:(ko + 1) * P], in0=ps[:],
                            scalar1=dinv_col[i][:, tt:tt + 1],
                            scalar2=0.0, op0=Alu.mult, op1=Alu.max)
                        if ko == TB - 1 or tt == TPC - 1:
                            tb = ko + 1
                            nc.sync.dma_start(
                                out=outs[i][ost_t0 * P:(ost_t0 + tb) * P, :]
                                .rearrange("(k p) f -> p k f", p=P),
                                in_=ost[:, :tb * P].rearrange(
                                    "p (k f) -> p k f", f=P))
                            ost = None

            # ---- pipelined emission ----
            dense_phase(0)
            lin_phase()
            dense_phase(1)
            edge_phase(0)
            dense_phase(2)
            edge_phase(1)
            edge_phase(2)
    nc.finalize()
    return nc


def _assemble(cfg, results):
    """results: list of per-core output dicts -> full outputs tuple."""
    N, A = cfg.N, cfg.A
    hs = []
    h0 = np.concatenate([results[k]["hs0"] for k in range(CORES)], axis=0)[:N]
    hs.append(h0.astype(np.float32))
    for i in range(A):
        o = np.concatenate([results[k][f"out{i}"] for k in range(CORES)], axis=0)[:N]
        hs.append(o.astype(np.float32))
    return tuple(hs)


def kernel(x, edge_index, edge_attr, lin_w, lin_b, conv_w, conv_b):
    global LAST_RESULTS
    x = np.asarray(x, np.float32)
    edge_index = np.asarray(edge_index)
    edge_attr = np.asarray(edge_attr, np.float32)
    lin_w = np.asarray(lin_w, np.float32)
    lin_b = np.asarray(lin_b, np.float32)
    conv_w = np.asarray(conv_w, np.float32)
    conv_b = np.asarray(conv_b, np.float32)

    N, D = x.shape
    A, _, E = edge_index.shape
    assert D == P
    cfg = _make_cfg(N, E, A)
    in_maps = _prep(cfg, x, edge_index, edge_attr, lin_w, lin_b, conv_w, conv_b)
    nc = _build(cfg)

    from concourse.bass_utils import run_bass_kernel_spmd
    res = run_bass_kernel_spmd(nc, in_maps, list(range(CORES)), trace=TRACE)
    LAST_RESULTS = res
    return _assemble(cfg, res.results)


# ---------- simulation path (for testing on small configs) ----------

def run_sim(x, edge_index, edge_attr, lin_w, lin_b, conv_w, conv_b,
            cores=None):
    """Run each core through CoreSim; returns assembled outputs."""
    from concourse import bass_interp
    x = np.asarray(x, np.float32)
    edge_index = np.asarray(edge_index)
    edge_attr = np.asarray(edge_attr, np.float32)
    N, D = x.shape
    A, _, E = edge_index.shape
    cfg = _make_cfg(N, E, A)
    in_maps = _prep(cfg, x, edge_index, edge_attr,
                    np.asarray(lin_w, np.float32), np.asarray(lin_b, np.float32),
                    np.asarray(conv_w, np.float32), np.asarray(conv_b, np.float32))
    results = []
    for k in (range(CORES) if cores is None else cores):
        nc = _build(cfg)
        sim = bass_interp.CoreSim(nc, core_id=0)
        sim.assign_tensors(in_maps[k])
        sim.simulate()
        results.append({name: sim.tensor(name).copy()
                        for name in ["hs0"] + [f"out{i}" for i in range(A)]})
    if cores is not None:
        return cfg, results
    return _assemble(cfg, results)


# revision 13
# speedup vs baseline: 2.1463x; 1.1354x over previous
"""GCN cascade layer (3 parallel GCNConv + 1 linear head) on 8 Trainium2 cores.

Math (per edge set i):
    deg[c]   = sum_{e: col=c} w[e]
    dinv     = deg>0 ? 1/sqrt(deg) : 0
    h        = x @ W_i.T
    out[c]   = relu( sum_e dinv[row]*w*dinv[c] * h[row] + b )
Reformulated so per-edge work is gather + weighted scatter only:
    h'[r]    = h[r] * dinv_m[r]            (dinv_m masked: 0 where deg==0)
    t[c]     = sum_{e->c} w[e] * h'[row[e]]
    out[c]   = relu( dinv_s[c] * (t[c] + s2[c]*b) )
  where dinv_s = 1/sqrt(max(deg,1-ish)) (==1 at deg==0), s2 = 1/dinv_s,
  so dinv_s*s2 == 1 and deg==0 columns give relu(b) exactly like the ref.

Distribution: output columns (nodes) sharded over 8 cores in contiguous
128-col tiles; h' computed replicated on every core (one pass over x per
set); per-core edges bucketed by 128-col tile on the host; gather of
h'[row] via the custom SWDGE dma_gather; scatter = one-hot-weighted matmul
accumulating into PSUM (exact segment sum on the TensorEngine).

Perf structure (v2): dma_gather descriptor generation runs on Q7 core pair
[2q, 2q+1] selected by queue_num — with num_swdge_queues=4 and round-robin
queue assignment, four gathers generate descriptors concurrently. Emission
is pipelined (dense_0, dense_1, edge_0, dense_2, edge_1, edge_2) so the PE
has dense work queued while the first gathers run, and DMAs are batched
4 tiles per transfer to cut sync-engine issue overhead.
"""

import sys

sys.path.insert(0, "/opt/trn_rl_repo")

import math
from dataclasses import dataclass, field

import numpy as np

import concourse.bass as bass
import concourse.bacc as bacc
import concourse.mybir as mybir
from concourse import tile, library_config

P = 128          # partitions / feature dim
CORES = 8
GATHER_GROUP = 7  # node tiles per dma_gather call group
NQ = 4            # SWDGE queues (gather desc-gen concurrency)
TB = 4            # tiles per batched DMA

f16 = mybir.dt.float16
f32 = mybir.dt.float32
i16 = mybir.dt.int16

# set by run(): BassKernelResults of the last hardware run (for profiling)
LAST_RESULTS = None
TRACE = False


@dataclass
class Cfg:
    N: int
    E: int
    A: int
    TPC: int          # node tiles per core
    NT: int           # total node tiles (CORES*TPC)
    N2: int           # padded node count (NT*P)
    OWN: int          # cols/rows owned per core (TPC*P)
    LO_T: int         # tiles in the "lo" half of h'
    LO_ROWS: int
    HI_T: int
    HI_ROWS: int
    S: int            # padded max col-degree slots
    K_lo: int         # lo chunks per node tile
    K_hi: int
    K_u: int          # K_lo + K_hi
    n_groups: int = 0
    group_tiles: list = field(default_factory=list)


def _make_cfg(N, E, A):
    TPC = math.ceil(N / (CORES * P))
    NT = CORES * TPC
    N2 = NT * P
    LO_T = (NT + 1) // 2
    LO_ROWS = LO_T * P
    HI_T = NT - LO_T
    HI_ROWS = HI_T * P
    assert LO_ROWS < 32768 and HI_ROWS < 32768, "int16 gather index overflow"
    cfg = Cfg(N=N, E=E, A=A, TPC=TPC, NT=NT, N2=N2, OWN=TPC * P,
              LO_T=LO_T, LO_ROWS=LO_ROWS, HI_T=HI_T, HI_ROWS=HI_ROWS,
              S=0, K_lo=0, K_hi=0, K_u=0)
    g = min(GATHER_GROUP, TPC)
    cfg.n_groups = math.ceil(TPC / g)
    cfg.group_tiles = [min(g, TPC - i * g) for i in range(cfg.n_groups)]
    return cfg


def _prep(cfg, x, edge_index, edge_attr, lin_w, lin_b, conv_w, conv_b):
    """Host-side sharding/layout prep. Returns the per-core input dict list."""
    A, N, E = cfg.A, cfg.N, cfg.E
    TPC, NT = cfg.TPC, cfg.NT

    r_all = edge_index[:, 0, :].astype(np.int64)   # [A,E]
    c_all = edge_index[:, 1, :].astype(np.int64)
    w_all = edge_attr.astype(np.float32)

    # --- degree (host only for LAYOUT: slot ranks + max degree) ---
    deg_counts = np.zeros((A, cfg.N2), np.int64)
    for i in range(A):
        deg_counts[i] = np.bincount(c_all[i], minlength=cfg.N2)
    S = int(deg_counts.max())
    S = max(S, 1)
    cfg.S = S

    # wdeg_all[i, p, tglob*S + s] = s-th weight of col (tglob*P + p)
    wdeg_all = np.zeros((A, P, NT * S), np.float16)
    wdeg_own = np.zeros((CORES, A, P, TPC * S), np.float16)

    # --- edge bucketing ---
    K_lo = K_hi = 0
    per_set = []
    for i in range(A):
        c = c_all[i]
        r = r_all[i]
        w = w_all[i]
        tile_of = c // P
        is_hi = (r >= cfg.LO_ROWS).astype(np.int64)
        order = np.lexsort((is_hi, tile_of))
        c_s, r_s, w_s, t_s, hi_s = c[order], r[order], w[order], tile_of[order], is_hi[order]
        seg_key = t_s * 2 + hi_s
        seg_change = np.empty(E, np.bool_)
        seg_change[0] = True
        seg_change[1:] = seg_key[1:] != seg_key[:-1]
        seg_start_idx = np.flatnonzero(seg_change)
        starts = np.zeros(E, np.int64)
        starts[seg_start_idx] = seg_start_idx
        starts = np.maximum.accumulate(starts)
        rank = np.arange(E) - starts
        n_lo = np.bincount(t_s[hi_s == 0], minlength=NT)
        n_hi = np.bincount(t_s[hi_s == 1], minlength=NT)
        K_lo = max(K_lo, int(math.ceil(n_lo.max() / P)))
        K_hi = max(K_hi, int(math.ceil(n_hi.max() / P)))
        per_set.append((c_s, r_s, w_s, t_s, hi_s, rank))

        # degree slot layout
        csort = np.sort(c)
        crank = np.arange(E) - np.maximum.accumulate(
            np.where(np.r_[True, csort[1:] != csort[:-1]], np.arange(E), 0))
        p_of = csort % P
        t_of = csort // P
        worder = np.argsort(c, kind="stable")
        wdeg_all[i, p_of, t_of * S + crank] = w[worder].astype(np.float16)
        for k in range(CORES):
            sel = (t_of >= k * TPC) & (t_of < (k + 1) * TPC)
            wdeg_own[k, i, p_of[sel], (t_of[sel] - k * TPC) * S + crank[sel]] = \
                w[worder][sel].astype(np.float16)

    K_lo = max(K_lo, 1)
    K_hi = max(K_hi, 1) if cfg.HI_T > 0 else 0
    cfg.K_lo, cfg.K_hi, cfg.K_u = K_lo, K_hi, K_lo + K_hi

    # --- per-core metadata arrays ---
    CH = TPC * cfg.K_u                     # chunks per core per set
    # bw[core, i, jj, chunk*P + c] = one-hot(col%P == c) * w, prebuilt on host
    bwall = np.zeros((CORES, A, P, CH * P), np.float16)
    gidx_lo = np.zeros((CORES, A, 16, TPC * K_lo * 8), np.int16)
    gidx_hi = np.zeros((CORES, A, 16, max(TPC * K_hi * 8, 1)), np.int16)

    for i in range(A):
        c_s, r_s, w_s, t_s, hi_s, rank = per_set[i]
        core = t_s // TPC
        tloc = t_s % TPC
        kk = rank // P          # chunk index within (tile, half)
        jj = rank % P           # partition
        lo_m = hi_s == 0
        col_idx = np.where(lo_m, tloc * cfg.K_u + kk, tloc * cfg.K_u + K_lo + kk)
        bwall[core, i, jj, col_idx * P + (c_s % P)] = w_s.astype(np.float16)
        gi = np.where(lo_m, r_s, r_s - cfg.LO_ROWS).astype(np.int16)
        pos = tloc * (np.where(lo_m, K_lo, K_hi) * P) + rank
        lo_sel = lo_m
        gidx_lo[core[lo_sel], i, pos[lo_sel] % 16, pos[lo_sel] // 16] = gi[lo_sel]
        if cfg.HI_T > 0:
            hi_sel = ~lo_m
            gidx_hi[core[hi_sel], i, pos[hi_sel] % 16, pos[hi_sel] // 16] = gi[hi_sel]

    # --- dense-phase inputs ---
    xpad = np.zeros((cfg.N2, P), np.float32)
    xpad[:N] = x
    xT_all = np.ascontiguousarray(xpad.T).astype(np.float16)        # [P, N2]
    WT = np.ascontiguousarray(conv_w.transpose(0, 2, 1)).astype(np.float16)  # [A,P,P]
    linWT = np.ascontiguousarray(lin_w.T).astype(np.float16)        # [P,P]
    lin_b_row = lin_b.reshape(1, P).astype(np.float16)
    ident = np.eye(P, dtype=np.float16)                              # [P,P]
    ones_row = np.ones((1, P), np.float16)
    b_bcast = np.tile(conv_b.reshape(A, 1, P), (1, P, 1)).astype(np.float16)  # [A,P,P]

    in_maps = []
    for k in range(CORES):
        m = dict(
            xT_all=xT_all,
            xT_own=np.ascontiguousarray(xT_all[:, k * cfg.OWN:(k + 1) * cfg.OWN]),
            wdeg_all=wdeg_all,
            wdeg_own=wdeg_own[k],
            WT=WT, linWT=linWT, lin_b_row=lin_b_row,
            ident=ident, ones_row=ones_row, b_bcast=b_bcast,
            bwall=bwall[k],
            gidx_lo=np.tile(gidx_lo[k], (1, 8, 1)),
        )
        if cfg.HI_T > 0:
            m["gidx_hi"] = np.tile(gidx_hi[k], (1, 8, 1))
        in_maps.append(m)
    return in_maps


def _build(cfg):
    """Build the single SPMD Bass program."""
    nc = bacc.Bacc(num_swdge_queues=NQ)
    A, TPC, NT, S = cfg.A, cfg.TPC, cfg.NT, cfg.S
    K_lo, K_hi, K_u = cfg.K_lo, cfg.K_hi, cfg.K_u
    CH = TPC * K_u
    Alu = mybir.AluOpType
    Act = mybir.ActivationFunctionType

    # ---- I/O ----
    xT_all = nc.dram_tensor("xT_all", [P, cfg.N2], f16, kind="ExternalInput")
    xT_own = nc.dram_tensor("xT_own", [P, cfg.OWN], f16, kind="ExternalInput")
    wdeg_all = nc.dram_tensor("wdeg_all", [A, P, NT * S], f16, kind="ExternalInput")
    wdeg_own = nc.dram_tensor("wdeg_own", [A, P, TPC * S], f16, kind="ExternalInput")
    WT = nc.dram_tensor("WT", [A, P, P], f16, kind="ExternalInput")
    linWT = nc.dram_tensor("linWT", [P, P], f16, kind="ExternalInput")
    lin_b_row = nc.dram_tensor("lin_b_row", [1, P], f16, kind="ExternalInput")
    ident_d = nc.dram_tensor("ident", [P, P], f16, kind="ExternalInput")
    ones_row = nc.dram_tensor("ones_row", [1, P], f16, kind="ExternalInput")
    b_bcast = nc.dram_tensor("b_bcast", [A, P, P], f16, kind="ExternalInput")
    bwall_d = nc.dram_tensor("bwall", [A, P, CH * P], f16, kind="ExternalInput")
    gidx_lo_d = nc.dram_tensor("gidx_lo", [A, 128, TPC * K_lo * 8], i16,
                               kind="ExternalInput")
    gidx_hi_d = (nc.dram_tensor("gidx_hi", [A, 128, TPC * K_hi * 8], i16,
                                kind="ExternalInput") if cfg.HI_T > 0 else None)

    hs0 = nc.dram_tensor("hs0", [cfg.OWN, P], f32, kind="ExternalOutput")
    outs = [nc.dram_tensor(f"out{i}", [cfg.OWN, P], f32, kind="ExternalOutput")
            for i in range(A)]

    h_lo = [nc.dram_tensor(f"h{i}_lo", [cfg.LO_ROWS, P], f16) for i in range(A)]
    h_hi = [nc.dram_tensor(f"h{i}_hi", [max(cfg.HI_ROWS, P), P], f16)
            for i in range(A)]

    qctr = [0]  # round-robin SWDGE queue counter

    with tile.TileContext(nc) as tc:
        with (
            tc.tile_pool(name="const", bufs=1) as cpool,
            tc.tile_pool(name="degio", bufs=2) as dpool,
            tc.tile_pool(name="degres", bufs=1) as rpool,
            tc.tile_pool(name="xw", bufs=3) as xpool,
            tc.tile_pool(name="hstage", bufs=3) as hpool,
            tc.tile_pool(name="glo", bufs=3) as glo_pool,
            tc.tile_pool(name="ghi", bufs=3) as ghi_pool,
            tc.tile_pool(name="gix", bufs=2) as gix_pool,
            tc.tile_pool(name="bw", bufs=3) as bwpool,
            tc.tile_pool(name="dg", bufs=2 * GATHER_GROUP + 1) as dgpool,
            tc.tile_pool(name="outst", bufs=3) as opool,
            tc.tile_pool(name="psd", bufs=4, space="PSUM") as psd,
            tc.tile_pool(name="pss", bufs=4, space="PSUM") as pss,
        ):
            # ---- constants to SBUF ----
            ident_t = cpool.tile([P, P], f16)
            nc.sync.dma_start(out=ident_t[:], in_=ident_d[:])
            ones_t = cpool.tile([1, P], f16)
            nc.sync.dma_start(out=ones_t[:], in_=ones_row[:])
            linb_t = cpool.tile([1, P], f16)
            nc.sync.dma_start(out=linb_t[:], in_=lin_b_row[:])
            linWT_t = cpool.tile([P, P], f16)
            nc.sync.dma_start(out=linWT_t[:], in_=linWT[:])
            WT_t = []
            b_t = []
            for i in range(A):
                wt = cpool.tile([P, P], f16, tag=f"WT{i}")
                nc.sync.dma_start(out=wt[:], in_=WT[i, :, :])
                WT_t.append(wt)
                bt = cpool.tile([P, P], f16, tag=f"bt{i}")
                nc.sync.dma_start(out=bt[:], in_=b_bcast[i, :, :])
                b_t.append(bt)

            # ---- degree phase ----
            def deg_pipeline(src, n_tiles, tag):
                deg = rpool.tile([P, n_tiles], f32, tag=f"deg_{tag}")
                GRP = max(1, min(n_tiles, 4096 // S))
                for g0 in range(0, n_tiles, GRP):
                    g1 = min(n_tiles, g0 + GRP)
                    wt = dpool.tile([P, GRP * S], f16, tag="degload")
                    nc.sync.dma_start(out=wt[:, :(g1 - g0) * S],
                                      in_=src[:, g0 * S:g1 * S])
                    nc.vector.tensor_reduce(
                        out=deg[:, g0:g1],
                        in_=wt[:, :(g1 - g0) * S].rearrange("p (g s) -> p g s", s=S),
                        axis=mybir.AxisListType.X, op=Alu.add)
                m = rpool.tile([P, n_tiles], f32, tag=f"m_{tag}")
                nc.vector.tensor_scalar(out=m[:], in0=deg[:], scalar1=0.0,
                                        scalar2=None, op0=Alu.is_gt)
                degsafe = rpool.tile([P, n_tiles], f32, tag=f"ds_{tag}")
                nc.vector.tensor_scalar(out=degsafe[:], in0=deg[:], scalar1=1.0,
                                        scalar2=None, op0=Alu.add)
                nc.vector.tensor_tensor(out=degsafe[:], in0=degsafe[:], in1=m[:],
                                        op=Alu.subtract)
                s = rpool.tile([P, n_tiles], f32, tag=f"s_{tag}")
                nc.scalar.activation(out=s[:], in_=degsafe[:], func=Act.Sqrt)
                dinv_safe = rpool.tile([P, n_tiles], f32, tag=f"dvs_{tag}")
                nc.vector.reciprocal(out=dinv_safe[:], in_=s[:])
                dinv_m = rpool.tile([P, n_tiles], f32, tag=f"dvm_{tag}")
                nc.vector.tensor_tensor(out=dinv_m[:], in0=dinv_safe[:], in1=m[:],
                                        op=Alu.mult)
                return m, dinv_m, dinv_safe, s

            # set 0 degrees first so dense_0 (and its gathers) unblock ASAP
            dinv_row = [None] * A   # [P, NT] f32 masked, per set
            dinv_col = [None] * A   # [P, TPC] f32 safe, per set
            s_col = [None] * A      # [P, TPC] f32 safe sqrt(deg), per set
            def deg_set(i):
                _, dm, _, _ = deg_pipeline(wdeg_all[i, :, :], NT, f"all{i}")
                dinv_row[i] = dm
                _, _, dvs, s_own = deg_pipeline(wdeg_own[i, :, :], TPC, f"own{i}")
                dinv_col[i] = dvs
                s_col[i] = s_own

            # ---- dense phase for one set: h'_i over all NT tiles, batched ----
            def dense_phase(i):
                for t0 in range(0, NT, TB):
                    tb = min(TB, NT - t0)
                    xt = xpool.tile([P, TB * P], f16, tag="xt")
                    nc.sync.dma_start(out=xt[:, :tb * P],
                                      in_=xT_all[:, t0 * P:(t0 + tb) * P])
                    hst = hpool.tile([P, TB * P], f16, tag="hst")
                    for k in range(tb):
                        t = t0 + k
                        ps = psd.tile([P, P], f32, tag="psd")
                        nc.tensor.matmul(out=ps[:], lhsT=xt[:, k * P:(k + 1) * P],
                                         rhs=WT_t[i][:], start=True, stop=True)
                        nc.scalar.activation(out=hst[:, k * P:(k + 1) * P],
                                             in_=ps[:], func=Act.Copy,
                                             scale=dinv_row[i][:, t:t + 1])
                    # batched write: SBUF [p, (k f)] -> DRAM rows t0*P..(t0+tb)*P
                    if t0 + tb <= cfg.LO_T:
                        dst = h_lo[i][t0 * P:(t0 + tb) * P, :]
                    else:
                        assert t0 >= cfg.LO_T, "TB must divide LO_T"
                        tt = t0 - cfg.LO_T
                        dst = h_hi[i][tt * P:(tt + tb) * P, :]
                    nc.scalar.dma_start(
                        out=dst.rearrange("(k p) f -> p k f", p=P),
                        in_=hst[:, :tb * P].rearrange("p (k f) -> p k f", f=P))

            # ---- hs0 (own rows only), batched ----
            def lin_phase():
                for t0 in range(0, TPC, TB):
                    tb = min(TB, TPC - t0)
                    xo = xpool.tile([P, TB * P], f16, tag="xo")
                    nc.sync.dma_start(out=xo[:, :tb * P],
                                      in_=xT_own[:, t0 * P:(t0 + tb) * P])
                    ost = opool.tile([P, TB * P], f32, tag="ost")
                    for k in range(tb):
                        ps = psd.tile([P, P], f32, tag="psd")
                        nc.tensor.matmul(out=ps[:], lhsT=xo[:, k * P:(k + 1) * P],
                                         rhs=linWT_t[:], start=True, stop=False)
                        nc.tensor.matmul(out=ps[:], lhsT=ones_t[:], rhs=linb_t[:],
                                         start=False, stop=True)
                        nc.scalar.activation(out=ost[:, k * P:(k + 1) * P],
                                             in_=ps[:], func=Act.Relu)
                    nc.sync.dma_start(
                        out=hs0[t0 * P:(t0 + tb) * P, :].rearrange(
                            "(k p) f -> p k f", p=P),
                        in_=ost[:, :tb * P].rearrange("p (k f) -> p k f", f=P))

            # shared gather-length registers (one MOVE each instead of 42)
            Lreg_cache = {}

            def Lreg(L):
                if L not in Lreg_cache:
                    Lreg_cache[L] = nc.gpsimd.to_reg(L)
                return Lreg_cache[L]

            # ---- edge phase for one set ----
            def edge_phase(i):
                # whole-set gather index loads (2 DMAs)
                gixlo = gix_pool.tile([128, TPC * K_lo * 8], i16, tag="gixlo")
                nc.sync.dma_start(out=gixlo[:], in_=gidx_lo_d[i, :, :])
                if cfg.HI_T > 0:
                    gixhi = gix_pool.tile([128, TPC * K_hi * 8], i16, tag="gixhi")
                    nc.sync.dma_start(out=gixhi[:], in_=gidx_hi_d[i, :, :])

                ost = None
                ost_t0 = 0
                for g in range(cfg.n_groups):
                    tg = cfg.group_tiles[g]
                    t0 = g * min(GATHER_GROUP, TPC)
                    GT = min(GATHER_GROUP, TPC)
                    # lo gather
                    glo = glo_pool.tile([P, GT * K_lo, P], f16, tag="glo")
                    L = tg * K_lo * P
                    off = t0 * K_lo * 8
                    nc.gpsimd.dma_gather(
                        out_ap=glo[:, :tg * K_lo, :], in_ap=h_lo[i][:, :],
                        idxs_ap=gixlo[:, off:off + L // 16],
                        num_idxs=L, num_idxs_reg=Lreg(L), elem_size=P,
                        single_packet=False, queue_num=qctr[0] % NQ)
                    qctr[0] += 1
                    # hi gather
                    if cfg.HI_T > 0:
                        ghi = ghi_pool.tile([P, GT * K_hi, P], f16, tag="ghi")
                        Lh = tg * K_hi * P
                        offh = t0 * K_hi * 8
                        nc.gpsimd.dma_gather(
                            out_ap=ghi[:, :tg * K_hi, :], in_ap=h_hi[i][:, :],
                            idxs_ap=gixhi[:, offh:offh + Lh // 16],
                            num_idxs=Lh, num_idxs_reg=Lreg(Lh), elem_size=P,
                            single_packet=False, queue_num=qctr[0] % NQ)
                        qctr[0] += 1
                    # bias diag tiles for the whole group up front so they are
                    # never queued behind a PSUM-waiting out-scale on the DVE
                    dgs = []
                    for tl in range(tg):
                        tt = t0 + tl
                        dg = dgpool.tile([P, P], f16, tag="dg")
                        nc.vector.tensor_scalar(
                            out=dg[:], in0=ident_t[:],
                            scalar1=s_col[i][:, tt:tt + 1],
                            scalar2=None, op0=Alu.mult)
                        dgs.append(dg)
                    for tl in range(tg):
                        tt = t0 + tl
                        if ost is None:
                            ost = opool.tile([P, TB * P], f32, tag="ost")
                            ost_t0 = tt
                        # host-prebuilt one-hot*w matrices for this tile
                        bwt = bwpool.tile([P, K_u * P], f16, tag="bw")
                        nc.sync.dma_start(
                            out=bwt[:],
                            in_=bwall_d[i, :, tt * K_u * P:(tt + 1) * K_u * P])
                        ps = pss.tile([P, P], f32, tag="pss")
                        for k in range(K_u):
                            if k < K_lo:
                                src = glo[:, tl * K_lo + k, :]
                            else:
                                src = ghi[:, tl * K_hi + (k - K_lo), :]
                            nc.tensor.matmul(out=ps[:],
                                             lhsT=bwt[:, k * P:(k + 1) * P],
                                             rhs=src,
                                             start=(k == 0), stop=False)
                        # bias: += diag(s2) @ b_bcast  ==> (t + s2*b)
                        nc.tensor.matmul(out=ps[:], lhsT=dgs[tl][:],
                                         rhs=b_t[i][:], start=False, stop=True)
                        ko = tt - ost_t0
                        nc.vector.tensor_scalar(
                            out=ost[:, ko * P:(ko + 1) * P], in0=ps[:],
                            scalar1=dinv_col[i][:, tt:tt + 1],
                            scalar2=0.0, op0=Alu.mult, op1=Alu.max)
                        if ko == TB - 1 or tt == TPC - 1:
                            tb = ko + 1
                            nc.sync.dma_start(
                                out=outs[i][ost_t0 * P:(ost_t0 + tb) * P, :]
                                .rearrange("(k p) f -> p k f", p=P),
                                in_=ost[:, :tb * P].rearrange(
                                    "p (k f) -> p k f", f=P))
                            ost = None

            # ---- pipelined emission ----
            deg_set(0)
            dense_phase(0)
            deg_set(1)
            deg_set(2)
            lin_phase()
            dense_phase(1)
            edge_phase(0)
            dense_phase(2)
            edge_phase(1)
            edge_phase(2)
    nc.finalize()
    return nc


def _assemble(cfg, results):
    """results: list of per-core output dicts -> full outputs tuple."""
    N, A = cfg.N, cfg.A
    hs = []
    h0 = np.concatenate([results[k]["hs0"] for k in range(CORES)], axis=0)[:N]
    hs.append(h0.astype(np.float32))
    for i in range(A):
        o = np.concatenate([results[k][f"out{i}"] for k in range(CORES)], axis=0)[:N]
        hs.append(o.astype(np.float32))
    return tuple(hs)


def kernel(x, edge_index, edge_attr, lin_w, lin_b, conv_w, conv_b):
    global LAST_RESULTS
    x = np.asarray(x, np.float32)
    edge_index = np.asarray(edge_index)
    edge_attr = np.asarray(edge_attr, np.float32)
    lin_w = np.asarray(lin_w, np.float32)
    lin_b = np.asarray(lin_b, np.float32)
    conv_w = np.asarray(conv_w, np.float32)
    conv_b = np.asarray(conv_b, np.float32)

    N, D = x.shape
    A, _, E = edge_index.shape
    assert D == P
    cfg = _make_cfg(N, E, A)
    in_maps = _prep(cfg, x, edge_index, edge_attr, lin_w, lin_b, conv_w, conv_b)
    nc = _build(cfg)

    from concourse.bass_utils import run_bass_kernel_spmd
    res = run_bass_kernel_spmd(nc, in_maps, list(range(CORES)), trace=TRACE)
    LAST_RESULTS = res
    return _assemble(cfg, res.results)


# ---------- simulation path (for testing on small configs) ----------

def run_sim(x, edge_index, edge_attr, lin_w, lin_b, conv_w, conv_b,
            cores=None):
    """Run each core through CoreSim; returns assembled outputs."""
    from concourse import bass_interp
    x = np.asarray(x, np.float32)
    edge_index = np.asarray(edge_index)
    edge_attr = np.asarray(edge_attr, np.float32)
    N, D = x.shape
    A, _, E = edge_index.shape
    cfg = _make_cfg(N, E, A)
    in_maps = _prep(cfg, x, edge_index, edge_attr,
                    np.asarray(lin_w, np.float32), np.asarray(lin_b, np.float32),
                    np.asarray(conv_w, np.float32), np.asarray(conv_b, np.float32))
    results = []
    for k in (range(CORES) if cores is None else cores):
        nc = _build(cfg)
        sim = bass_interp.CoreSim(nc, core_id=0)
        sim.assign_tensors(in_maps[k])
        sim.simulate()
        results.append({name: sim.tensor(name).copy()
                        for name in ["hs0"] + [f"out{i}" for i in range(A)]})
    if cores is not None:
        return cfg, results
    return _assemble(cfg, results)
